# revision 20
# baseline (speedup 1.0000x reference)
"""Bidirectional Mamba block (nn_Block_bi_mamba) Trainium2 Bass kernel.

Sharding: 8 cores = (batch b in {0,1}) x (d_inner quarter dq in {0..3}).
Each core computes, for its batch and both scan directions, the full
in_proj+conv (folded into PE matmuls) and x_proj (contracts over all 512
channels), the selective scan for its own 128 channels, and the out_proj
partial product [256, L]. The host sums the 4 partials per batch and
adds the residual x. The d_inner axis is permuted per core so the core's
own channel block is always channel-tile 0, making the device program
identical across cores (SPMD) with only input data differing.

v2 changes vs the 653us baseline:
- The conv-folded in_proj and the z-projection run as fp8e4m3 DoubleRow
  matmuls (256-deep contraction, 0.5 cyc/col): ~4x less PE time. Host
  pre-scales x by 16 and the folded weights by a power-of-2 to fill the
  e4m3 range; the descale rides the (free) activation scale. End-to-end
  error stays ~2e-4 because attn_out is small vs the residual x.
- silu is the native Silu activation applied directly to conv PSUM
  (fused bias + fp8 descale + nonlinearity in one scalar op); this also
  drops the sigmoid table (2 ACT_TABLE_LOADs per combo instead of ~12)
  and the DVE silu multiplies.
- K state-pairs per combo run their scans on GpSimd instead of the DVE
  (both engines implement tensor_tensor_scan); the DVE keeps the
  elementwise dBu/hC multiplies it is uniquely fast at (2x fp16 mode).

Device layout: d-major [128 chans, time]. The SSM recurrence runs on
DVE/GpSimd tensor_tensor_scan (fp16). The y = sum_n h_n*C_n reduction,
the D*u skip term, and the y_f + y_b combine ride the Tensor engine as
identity/diagonal matmuls accumulating into a PSUM tile (high_priority
so they never queue behind the next combo's conv matmuls). B/C rows
broadcast across partitions via DRAM-source stride-0 DMA, two states
per transfer, with one wide in-place DVE multiply per pair (du repeated
via a stride-0 AP). The per-(dir,chunk) front-end is software-pipelined
one step ahead of the scan phase.

Self-contained: hardcodes all shapes; no sibling imports.
"""
import os
import numpy as np
import ml_dtypes
from contextlib import ExitStack

import concourse.bacc as bacc
import concourse.bass as bass
import concourse.tile as tile
from concourse import mybir
from concourse.bass_utils import run_bass_kernel_spmd

bf = ml_dtypes.bfloat16
f8 = ml_dtypes.float8_e4m3
FP32 = mybir.dt.float32
BF16 = mybir.dt.bfloat16
FP16 = mybir.dt.float16
FP8 = mybir.dt.float8e4

B, L = 2, 4096
LC = 2048
NCH = L // LC
NSUB = LC // 512
N = 16
XPAD = L + 6          # fp8 x tile: 3 left pad + L + 3 right pad
AOP = mybir.AluOpType
AF = mybir.ActivationFunctionType
DR = mybir.MatmulPerfMode.DoubleRow

S_X = 16.0            # host pre-scale on x for fp8
# np_ pairs (0..7) whose dBu and h*C elementwise multiplies run on GpSimd.
# Default OFF: measured on HW, co-running Pool with the DVE slows DVE scans
# ~2.7x (SBUF port contention) and Pool TT itself is ~4x slower than DVE.
POOL_TT = tuple(
    int(t) for t in os.environ.get("BMK_POOL_TT", "").split(",") if t != ""
)


def _bcast_from_dram(nc, dst, row):
    """DMA-broadcast a [1, F] DRAM row across all partitions of dst."""
    rap = [list(x) for x in row.ap]
    src = bass.AP(tensor=row.tensor, offset=row.offset,
                  ap=[[0, dst.shape[0]], rap[1]])
    nc.sync.dma_start(out=dst, in_=src)


def _view3(t2, dim1, dim2):
    """Reshape a 2D slice AP into [part, dim1, dim2] (strides in elems)."""
    return bass.AP(tensor=t2.tensor, offset=t2.offset,
                   ap=[list(t2.ap[0]), list(dim1), list(dim2)])


def build_program(tc, ins, outs):
    nc = tc.nc
    with ExitStack() as ctx:
        wp = ctx.enter_context(tc.tile_pool(name="wp", bufs=1))
        big = ctx.enter_context(tc.tile_pool(name="big", bufs=1))
        work = ctx.enter_context(tc.tile_pool(name="work", bufs=1))
        scanp = ctx.enter_context(tc.tile_pool(name="scanp", bufs=2))
        ps = ctx.enter_context(tc.tile_pool(name="ps", bufs=1, space="PSUM"))
        dramp = ctx.enter_context(tc.tile_pool(name="dramp", bufs=3,
                                               space="DRAM"))

        # ---- weights ----
        # wconst fp32 [128, 76]: An_f 0:16 | An_b 16:32 | (unused) |
        #   convb_f 64:68 | convb_b 68:72 | dtbias_f 74 | dtbias_b 75
        wconst = wp.tile([128, 76], FP32, tag="wconst")
        nc.sync.dma_start(out=wconst, in_=ins["wconst"])
        COL = {"An_f": 0, "An_b": 16, "cb_f": 64, "cb_b": 68,
               "dtb_f": 74, "dtb_b": 75}

        # fp8 conv weights: one [128, 1024] slab per (dir, mt);
        # slab col layout: tap k -> [k*256 : k*256+256] = [kt0 128][kt1 128]
        wconv = wp.tile([128, 8 * 1024], FP8, tag="wconv")
        # load b-dir slabs first: the first combo is b-direction.
        for dcol_mt in ([4 + m for m in range(4)] + list(range(4))):
            nc.sync.dma_start(
                out=wconv[:, dcol_mt * 1024:(dcol_mt + 1) * 1024],
                in_=ins["wconvP8"][:, dcol_mt * 1024:(dcol_mt + 1) * 1024])
        wz = wp.tile([128, 256], FP8, tag="wz")
        outw = wp.tile([128, 256], FP16, tag="outw")
        xpro = wp.tile([128, 384], FP16, tag="xpro")
        dtprojp = wp.tile([16, 256], FP16, tag="dtprojp")
        ident = wp.tile([128, 128], BF16, tag="ident")
        diagD = wp.tile([128, 256], BF16, tag="diagD")
        carry = wp.tile([128, 32], FP32, tag="carry")

        def _late_weight_dmas():
            nc.sync.dma_start(out=wz, in_=ins["wzP8"])
            nc.sync.dma_start(out=outw, in_=ins["outWT"])
            nc.sync.dma_start(out=xpro, in_=ins["xprojP"])
            nc.sync.dma_start(out=dtprojp, in_=ins["dtprojp"])
            nc.sync.dma_start(out=ident, in_=ins["ident"])
            nc.sync.dma_start(out=diagD, in_=ins["diagD"])

        # ---- persistent buffers ----
        # x in fp8: [128, 2*XPAD]; col j*XPAD + 3 + t = x[t] for model dims
        # 128*j + p, pre-scaled by S_X. First combo reads chunk 1 (b-dir),
        # so load the second half of time first.
        x8 = big.tile([128, 2 * XPAD], FP8, tag="x8", name="x8")
        HALF = 3 + LC
        for j in range(2):
            nc.sync.dma_start(
                out=x8[:, j * XPAD + HALF:(j + 1) * XPAD],
                in_=ins["x8p"][:, j * XPAD + HALF:(j + 1) * XPAD])
        _late_weight_dmas()
        for j in range(2):
            nc.sync.dma_start(
                out=x8[:, j * XPAD:j * XPAD + HALF],
                in_=ins["x8p"][:, j * XPAD:j * XPAD + HALF])
        zs_all = big.tile([128, L], FP16, tag="zs")
        y_ball = big.tile([128, L], FP16, tag="yball")

        DSC = {"f": ins["scales"]["f"], "b": ins["scales"]["b"],
               "z": ins["scales"]["z"]}

        def _conv_mm(pt, dcol, mt, ns0):
            """4 fp8 DoubleRow matmuls accumulating conv+in_proj into pt."""
            slab = (dcol * 4 + mt) * 1024
            for k in range(4):
                w2 = wconv[:, slab + k * 256:slab + k * 256 + 256]
                lhsT = _view3(w2, [128, 2], [1, 128])
                xs = x8[:, ns0 + k:ns0 + k + 512]
                rhs = _view3(xs, [XPAD, 2], [1, 512])
                nc.tensor.matmul(pt, lhsT, rhs, start=(k == 0),
                                 stop=(k == 3), perf_mode=DR)

        # ---- phase Z ----
        def phase_z(c):
            for nsub in range(NSUB):
                pt = ps.tile([128, 512], FP32, tag="ps_conv", bufs=2)
                z2 = _view3(wz[:, 0:256], [128, 2], [1, 128])
                xs = x8[:, 3 + c * LC + nsub * 512:3 + c * LC + nsub * 512 + 512]
                rhs = _view3(xs, [XPAD, 2], [1, 512])
                nc.tensor.matmul(pt, z2, rhs, start=True, stop=True,
                                 perf_mode=DR)
                nc.scalar.activation(
                    out=zs_all[:, c * LC + nsub * 512:c * LC + (nsub + 1) * 512],
                    in_=pt, func=AF.Silu, bias=0.0, scale=DSC["z"])

        combos = ([("b", c) for c in range(NCH - 1, -1, -1)]
                  + [("f", c) for c in range(NCH)])

        def front_end(d, c, fine=False):
            """conv -> Silu(psum*descale+bias) -> xc; x_proj -> dbl; dt.

            nsub-major order with per-half (1024 col) Exp/Ln/du so the dt
            chain is ready half-way into the front-end. For b-dirs the C
            rows are stored time-reversed in dbl (via reversed scalar
            copies) so the scan phase can use one wide h*C multiply.
            fine=True (first combo only): process nsubs in reverse time
            order so the half the backward scan needs first is ready first.
            """
            dcol = 0 if d == "f" else 1
            base = 0 if d == "f" else 3
            cb0 = COL[f"cb_{d}"]
            xc = [work.tile([128, LC], FP16,
                            tag=(f"xc0{d}" if t == 0 else f"xc{t}"),
                            name=f"xc{t}", bufs=2)
                  for t in range(4)]
            dbl = work.tile([48, LC], FP16, tag="dbl", bufs=2)
            vsub = work.tile([128, LC], FP16, tag="vsub", bufs=2)
            dt = work.tile([128, LC], FP16, tag="dt", bufs=2)
            esub = work.tile([128, LC], FP16, tag="esub", bufs=2)
            du = work.tile([128, LC], FP16, tag="du", bufs=2)
            scratch = dramp.tile([32, LC], FP16, tag="bcdram")

            ns_order = [3, 2, 1, 0] if fine else [0, 1, 2, 3]
            for step, nsub in enumerate(ns_order):
                ns0 = c * LC + nsub * 512 + base
                for mt in range(4):
                    pt = ps.tile([128, 512], FP32, tag="ps_conv", bufs=2)
                    _conv_mm(pt, dcol, mt, ns0)
                    nc.scalar.activation(
                        out=xc[mt][:, nsub * 512:(nsub + 1) * 512], in_=pt,
                        func=AF.Silu,
                        bias=wconst[:, cb0 + mt:cb0 + mt + 1],
                        scale=DSC[d])
                pj = ps.tile([48, 512], FP32, tag="ps_small", name="pj", bufs=2)
                for kt in range(4):
                    nc.tensor.matmul(
                        pj, xpro[:, kt * 96 + 48 * dcol:
                                 kt * 96 + 48 * (dcol + 1)],
                        xc[kt][:, nsub * 512:(nsub + 1) * 512],
                        start=(kt == 0), stop=(kt == 3))
                sl = slice(nsub * 512, (nsub + 1) * 512)
                if d == "b":
                    # store C rows time-reversed: aligned with the
                    # reversed-stored h2 so h*C is one wide multiply
                    nc.scalar.copy(out=dbl[0:32, sl], in_=pj[0:32, :])
                    rsl = slice((3 - nsub) * 512, (4 - nsub) * 512)
                    nc.scalar.copy(out=dbl[32:48, rsl][:, ::-1],
                                   in_=pj[32:48, :])
                else:
                    nc.scalar.copy(out=dbl[:, sl], in_=pj)
                ptdt = ps.tile([128, 512], FP32, tag="ps_small", name="ptdt", bufs=2)
                nc.tensor.matmul(
                    ptdt, dtprojp[:, dcol * 128:(dcol + 1) * 128],
                    dbl[0:16, sl], start=True, stop=True)
                # Identity (+dt bias) lives in every act table: no load
                nc.scalar.activation(
                    out=vsub[:, sl], in_=ptdt, func=AF.Identity,
                    bias=wconst[:, COL[f"dtb_{d}"]:COL[f"dtb_{d}"] + 1],
                    scale=1.0)
                if step % 2 == 1:
                    # this half's nsubs are done: softplus = ln(1+exp(v))
                    # and du for the half, so the scan inputs for the next
                    # combo (or the fine fill) are ready early
                    h0 = min(nsub, ns_order[step - 1]) * 512
                    hs = slice(h0, h0 + 1024)
                    nc.scalar.activation(out=esub[:, hs], in_=vsub[:, hs],
                                         func=AF.Exp, bias=0.0, scale=1.0)
                    nc.scalar.activation(out=dt[:, hs], in_=esub[:, hs],
                                         func=AF.Ln, bias=1.0, scale=1.0)
                    nc.vector.tensor_tensor(du[:, hs], dt[:, hs],
                                            xc[0][:, hs], AOP.mult)
                    if fine:
                        # early half-DMA of the B rows only (C rows for
                        # this half live in not-yet-written dbl columns)
                        nc.sync.dma_start(out=scratch[0:16, hs],
                                          in_=dbl[16:32, hs])
            if fine:
                nc.sync.dma_start(out=scratch[16:32, :], in_=dbl[32:48, :])
            else:
                nc.sync.dma_start(out=scratch, in_=dbl[16:48, :])
            return {"xc0": xc[0], "dt": dt, "du": du, "scratch": scratch}

        ub_store = {}

        def scan_phase(d, c, st, first, fine=False):
            dcol = 0 if d == "f" else 1
            rev = (lambda ap: ap[:, ::-1]) if d == "b" else (lambda ap: ap)
            dt, du, scratch, u = st["dt"], st["du"], st["scratch"], st["xc0"]
            if d == "b":
                ub_store[c] = u

            psy = ps.tile([128, LC], FP32, tag="ps_y")

            def bcast_mul(dst2, rows, mul, h0, w):
                """One DMA broadcasting two scratch row-segments [h0:h0+w]
                into both state-halves of dst2, then one in-place multiply
                dst2 *= repeat(mul[:, h0:h0+w], 2)."""
                rs = rows[:, h0:h0 + w]
                rap = [list(x) for x in rs.ap]
                src = bass.AP(tensor=rs.tensor, offset=rs.offset,
                              ap=[[0, 128], rap[0], rap[1]])
                d3 = bass.AP(tensor=dst2.tensor, offset=dst2.offset + h0,
                             ap=[list(dst2.ap[0]), [LC, 2], [1, w]])
                nc.sync.dma_start(out=d3, in_=src)
                mrep = bass.AP(tensor=mul.tensor, offset=mul.offset + h0,
                               ap=[list(mul.ap[0]), [0, 2], [1, w]])
                nc.vector.tensor_tensor(d3, mrep, d3, AOP.mult)

            # fine (first combo, d=="b"): dA/bbc/scan at half-chunk
            # granularity, sub-scans chained via initial, so the first scan
            # starts as soon as the front-end's first half is done.
            H = LC // 2
            for np_ in range(N // 2):
                n0 = 2 * np_
                dA2 = scanp.tile([128, 2 * LC], FP16, tag="dA", bufs=2)
                bbc2 = scanp.tile([128, 2 * LC], FP16, tag="bbc", bufs=2)
                h2 = scanp.tile([128, 2 * LC], FP16, tag="h", bufs=3)
                if fine:
                    for h0 in (H, 0):  # backward scan: late half first
                        for i in range(2):
                            nc.scalar.activation(
                                out=dA2[:, i * LC + h0:i * LC + h0 + H],
                                in_=dt[:, h0:h0 + H], func=AF.Exp, bias=0.0,
                                scale=wconst[:, COL[f"An_{d}"] + n0 + i:
                                             COL[f"An_{d}"] + n0 + i + 1])
                        bcast_mul(bbc2, scratch[n0:n0 + 2, :], du, h0, H)
                    for i in range(2):
                        # sub-scan A: times [H, LC) reversed -> h2[0:H)
                        nc.vector.tensor_tensor_scan(
                            h2[:, i * LC:i * LC + H],
                            dA2[:, i * LC + H:(i + 1) * LC][:, ::-1],
                            bbc2[:, i * LC + H:(i + 1) * LC][:, ::-1],
                            0.0, AOP.mult, AOP.add)
                        # sub-scan B: times [0, H) reversed, chained
                        nc.vector.tensor_tensor_scan(
                            h2[:, i * LC + H:(i + 1) * LC],
                            dA2[:, i * LC:i * LC + H][:, ::-1],
                            bbc2[:, i * LC:i * LC + H][:, ::-1],
                            h2[:, i * LC + H - 1:i * LC + H],
                            AOP.mult, AOP.add)
                        if NCH > 1:
                            n = n0 + i
                            nc.vector.tensor_copy(
                                out=carry[:, dcol * 16 + n:dcol * 16 + n + 1],
                                in_=h2[:, (i + 1) * LC - 1:(i + 1) * LC])
                else:
                    for i in range(2):
                        nc.scalar.activation(
                            out=dA2[:, i * LC:(i + 1) * LC], in_=dt,
                            func=AF.Exp, bias=0.0,
                            scale=wconst[:, COL[f"An_{d}"] + n0 + i:
                                         COL[f"An_{d}"] + n0 + i + 1])
                    bcast_mul(bbc2, scratch[n0:n0 + 2, :], du, 0, LC)
                    for i in range(2):
                        n = n0 + i
                        hsl = h2[:, i * LC:(i + 1) * LC]
                        init = (0.0 if first
                                else carry[:, dcol * 16 + n:dcol * 16 + n + 1])
                        nc.vector.tensor_tensor_scan(
                            hsl, rev(dA2[:, i * LC:(i + 1) * LC]),
                            rev(bbc2[:, i * LC:(i + 1) * LC]), init,
                            AOP.mult, AOP.add)
                        if first and NCH > 1:
                            nc.vector.tensor_copy(
                                out=carry[:, dcol * 16 + n:dcol * 16 + n + 1],
                                in_=hsl[:, LC - 1:LC])
                cbc2 = scanp.tile([128, 2 * LC], FP16, tag="cbc", bufs=2)
                rap = [list(x) for x in scratch[16 + n0:18 + n0, :].ap]
                src = bass.AP(tensor=scratch.tensor,
                              offset=scratch[16 + n0:18 + n0, :].offset,
                              ap=[[0, 128], rap[0], rap[1]])
                d3 = bass.AP(tensor=cbc2.tensor, offset=cbc2.offset,
                             ap=[list(cbc2.ap[0]), [LC, 2], [1, LC]])
                nc.sync.dma_start(out=d3, in_=src)
                # C rows are stored pre-reversed for b-dirs: one wide
                # multiply either way
                h3 = bass.AP(tensor=h2.tensor, offset=h2.offset,
                             ap=[list(h2.ap[0]), [LC, 2], [1, LC]])
                c3 = bass.AP(tensor=cbc2.tensor, offset=cbc2.offset,
                             ap=[list(cbc2.ap[0]), [LC, 2], [1, LC]])
                nc.vector.tensor_tensor(h3, h3, c3, AOP.mult)
                with tc.high_priority():
                    for i in range(2):
                        n = n0 + i
                        for q in range(NSUB):
                            nc.tensor.matmul(
                                psy[:, q * 512:(q + 1) * 512], ident,
                                h2[:, i * LC + q * 512:i * LC + (q + 1) * 512],
                                start=(n == 0),
                                stop=(d == "b" and n == N - 1),
                                skip_group_check=True)

            if d == "b":
                # The D_b*u_b skip term rides the matching f-combo's PSUM.
                # psy holds y_b in reversed time; un-reverse on copy-out.
                for q in range(NSUB):
                    fseg = 3 - q
                    nc.scalar.copy(
                        out=y_ball[:, c * LC + fseg * 512:
                                   c * LC + (fseg + 1) * 512][:, ::-1],
                        in_=psy[:, q * 512:(q + 1) * 512])
            else:
                u_b = ub_store[c]
                for q in range(NSUB):
                    sl = slice(q * 512, (q + 1) * 512)
                    nc.tensor.matmul(psy[:, sl], diagD[:, 0:128], u[:, sl],
                                     start=False, stop=False,
                                     skip_group_check=True)
                    nc.tensor.matmul(psy[:, sl], diagD[:, 128:256],
                                     u_b[:, sl], start=False, stop=False,
                                     skip_group_check=True)
                    nc.tensor.matmul(
                        psy[:, sl], ident,
                        y_ball[:, c * LC + q * 512:c * LC + (q + 1) * 512],
                        start=False, stop=True, skip_group_check=True)
                ysum = work.tile([128, LC], FP16, tag="ysum", bufs=1)
                ygated = work.tile([128, LC], FP16, tag="ygated", bufs=1)
                with tc.high_priority():
                    for q in range(NSUB):
                        sl = slice(q * 512, (q + 1) * 512)
                        nc.scalar.copy(out=ysum[:, sl], in_=psy[:, sl])
                        nc.vector.tensor_tensor(
                            ygated[:, sl], ysum[:, sl],
                            zs_all[:, c * LC + q * 512:
                                   c * LC + (q + 1) * 512],
                            AOP.mult)
                for mt in range(2):
                    osb = work.tile([128, LC], FP32, tag="osb", bufs=1)
                    for nsub in range(NSUB):
                        po = ps.tile([128, 512], FP32, tag="ps_small", name="po", bufs=2)
                        nc.tensor.matmul(
                            po, outw[:, mt * 128:(mt + 1) * 128],
                            ygated[:, nsub * 512:(nsub + 1) * 512],
                            start=True, stop=True)
                        nc.scalar.copy(
                            out=osb[:, nsub * 512:(nsub + 1) * 512], in_=po)
                        nc.sync.dma_start(
                            out=outs["attnT"][mt * 128:(mt + 1) * 128,
                                              c * LC + nsub * 512:
                                              c * LC + (nsub + 1) * 512],
                            in_=osb[:, nsub * 512:(nsub + 1) * 512])

        # software pipeline: front_end one combo ahead of the scan phase;
        # phase-Z rides in the shadow of the first front-end
        states = {}
        states[0] = front_end(*combos[0], fine=True)
        for j, (d, c) in enumerate(combos):
            if j + 1 < len(combos):
                # Gate the next combo's front-end behind the fill window so
                # its scalar/PE ops don't wedge into combo j's critical
                # chain on the in-order engines.
                with tc.tile_wait_until(0.045 if j == 0 else 0):
                    states[j + 1] = front_end(*combos[j + 1])
            if j == 1:
                with tc.tile_wait_until(0.110):
                    for c2 in range(NCH):
                        phase_z(c2)
            first = (j % NCH == 0)
            scan_phase(d, c, states.pop(j), first, fine=(j == 0))


def build_nc(scales):
    nc = bacc.Bacc("TRN2", target_bir_lowering=False, debug=False,
                   enable_asserts=False)
    ins = {}

    def inp(name, shape, dt):
        ins[name] = nc.dram_tensor(name, shape, dt,
                                   kind="ExternalInput").ap()

    inp("x8p", [128, 2 * XPAD], FP8)
    inp("wconvP8", [128, 8 * 1024], FP8)
    inp("wzP8", [128, 256], FP8)
    inp("outWT", [128, 256], FP16)
    inp("xprojP", [128, 384], FP16)
    inp("dtprojp", [16, 256], FP16)
    inp("wconst", [128, 76], FP32)
    inp("ident", [128, 128], BF16)
    inp("diagD", [128, 256], BF16)
    ins["scales"] = scales
    outs = {"attnT": nc.dram_tensor("attnT", [256, L], FP32,
                                    kind="ExternalOutput").ap()}
    with tile.TileContext(nc) as tc:
        build_program(tc, ins, outs)
    nc.compile()
    return nc


def _pow2_scale(maxabs, target=192.0):
    """Largest power of 2 s with maxabs*s <= target (e4m3 max 240)."""
    import math
    if maxabs <= 0:
        return 1.0
    return 2.0 ** math.floor(math.log2(target / maxabs))


def prep_scales(inputs):
    """Power-of-2 fp8 pre-scales shared by all cores (weight-dependent)."""
    ipw = inputs["in_proj_w"].astype(np.float64)
    scales = {}
    for d in "fb":
        cw = inputs[f"conv_w_{d}"][:, 0, :].astype(np.float64)
        wmax = (np.abs(cw).max(axis=1)[:, None]
                * np.abs(ipw[:512]).max(axis=1)[:, None]).max()
        # bound on |tap_k * w_inx| entries
        wmax = max((np.abs(cw)[:, :, None]
                    * np.abs(ipw[:512])[:, None, :]).max(), 1e-12)
        scales[d] = _pow2_scale(wmax)
    scales["z"] = _pow2_scale(np.abs(ipw[512:]).max())
    return scales


_CACHE = {}


def prep_core_inputs(inputs, b, dq, scales):
    """Per-core input arrays; d_inner axis permuted so own block is first."""
    own = np.arange(dq * 128, (dq + 1) * 128)
    rest = np.array([i for i in range(512)
                     if not (dq * 128 <= i < (dq + 1) * 128)])
    perm = np.concatenate([own, rest])

    out = {}
    xT = inputs["x"][b].T.astype(np.float32)  # [256, L]
    x8p = np.zeros((128, 2 * XPAD), np.float32)
    for j in range(2):
        x8p[:, j * XPAD + 3:j * XPAD + 3 + L] = xT[j * 128:(j + 1) * 128] * S_X
    out["x8p"] = x8p.astype(f8)

    w_inx = inputs["in_proj_w"][:512][perm].astype(np.float64)  # [512, 256]
    wconvP = np.zeros((128, 8 * 1024), np.float64)
    for dcol, d in enumerate("fb"):
        cw = inputs[f"conv_w_{d}"][:, 0, :][perm].astype(np.float64)
        sw = scales[d]
        for k in range(4):
            tap = cw[:, k] if d == "f" else cw[:, 3 - k]
            WdkT = (tap[:, None] * w_inx).T * sw     # [256, 512]
            for mt in range(4):
                slab = (dcol * 4 + mt) * 1024
                for kt in range(2):
                    off = slab + k * 256 + kt * 128
                    wconvP[:, off:off + 128] = \
                        WdkT[kt * 128:(kt + 1) * 128,
                             mt * 128:(mt + 1) * 128]
    out["wconvP8"] = wconvP.astype(f8)

    wz = inputs["in_proj_w"][512:1024][own].astype(np.float64)  # [128, 256]
    wzP = np.zeros((128, 256), np.float64)
    for kt in range(2):
        wzP[:, kt * 128:(kt + 1) * 128] = wz.T[kt * 128:(kt + 1) * 128]
    out["wzP8"] = (wzP * scales["z"]).astype(f8)

    out["outWT"] = np.ascontiguousarray(
        inputs["out_proj_w"][:, own].T).astype(np.float16)  # [128, 256]

    xprojP = np.zeros((128, 384), np.float32)
    xpf = inputs["xproj_w_f"][:, perm].T  # [512, 48]
    xpb = inputs["xproj_w_b"][:, perm].T
    for kt in range(4):
        xprojP[:, kt * 96:kt * 96 + 48] = xpf[kt * 128:(kt + 1) * 128]
        xprojP[:, kt * 96 + 48:kt * 96 + 96] = xpb[kt * 128:(kt + 1) * 128]
    out["xprojP"] = xprojP.astype(np.float16)

    out["dtprojp"] = np.ascontiguousarray(np.concatenate(
        [inputs["dtproj_w_f"][own].T, inputs["dtproj_w_b"][own].T],
        axis=1)).astype(np.float16)  # [16, 256]

    wconst = np.zeros((128, 76), np.float32)
    for i, d in enumerate("fb"):
        wconst[:, 16 * i:16 * i + 16] = -np.exp(
            inputs[f"A_log_{d}"][own].astype(np.float64))
        cb = inputs[f"conv_b_{d}"][perm]
        wconst[:, 64 + 4 * i:68 + 4 * i] = cb.reshape(4, 128).T
        wconst[:, 74 + i] = inputs[f"dtproj_b_{d}"][own]
    out["wconst"] = wconst

    out["ident"] = np.eye(128, dtype=np.float32).astype(bf)
    diagD = np.zeros((128, 256), np.float32)
    diagD[:, 0:128] = np.diag(inputs["D_f"][own])
    diagD[:, 128:256] = np.diag(inputs["D_b"][own])
    out["diagD"] = diagD.astype(bf)
    return out


def kernel(**inputs):
    inputs = {k: np.asarray(v) for k, v in inputs.items()}
    scales = prep_scales(inputs)
    descales = {"f": 1.0 / (S_X * scales["f"]),
                "b": 1.0 / (S_X * scales["b"]),
                "z": 1.0 / (S_X * scales["z"])}
    if "nc" not in _CACHE:
        _CACHE["nc"] = build_nc(descales)
    nc = _CACHE["nc"]

    core_ids = list(range(8))
    in_maps = [prep_core_inputs(inputs, core // 4, core % 4, scales)
               for core in core_ids]
    trace = os.environ.get("BASS_KERNEL_TRACE", "0") == "1"
    res = run_bass_kernel_spmd(nc, in_maps, core_ids, trace=trace)
    _CACHE["last_results"] = res

    x = inputs["x"].astype(np.float32)
    out = np.empty((B, L, 256), np.float32)
    for b in range(B):
        acc = np.zeros((256, L), np.float32)
        for dq in range(4):
            acc += res.results[4 * b + dq]["attnT"]
        out[b] = x[b] + acc.T
    return out.astype(np.float32)


# revision 24
# speedup vs baseline: 1.0308x; 1.0308x over previous
"""Bidirectional Mamba block (nn_Block_bi_mamba) Trainium2 Bass kernel.

Sharding: 8 cores = (batch b in {0,1}) x (d_inner quarter dq in {0..3}).
Each core computes, for its batch and both scan directions, the full
in_proj+conv (folded into PE matmuls) and x_proj (contracts over all 512
channels), the selective scan for its own 128 channels, and the out_proj
partial product [256, L]. The host sums the 4 partials per batch and
adds the residual x. The d_inner axis is permuted per core so the core's
own channel block is always channel-tile 0, making the device program
identical across cores (SPMD) with only input data differing.

v2 changes vs the 653us baseline:
- The conv-folded in_proj and the z-projection run as fp8e4m3 DoubleRow
  matmuls (256-deep contraction, 0.5 cyc/col): ~4x less PE time. Host
  pre-scales x by 16 and the folded weights by a power-of-2 to fill the
  e4m3 range; the descale rides the (free) activation scale. End-to-end
  error stays ~2e-4 because attn_out is small vs the residual x.
- silu is the native Silu activation applied directly to conv PSUM
  (fused bias + fp8 descale + nonlinearity in one scalar op); this also
  drops the sigmoid table (2 ACT_TABLE_LOADs per combo instead of ~12)
  and the DVE silu multiplies.
- K state-pairs per combo run their scans on GpSimd instead of the DVE
  (both engines implement tensor_tensor_scan); the DVE keeps the
  elementwise dBu/hC multiplies it is uniquely fast at (2x fp16 mode).

Device layout: d-major [128 chans, time]. The SSM recurrence runs on
DVE/GpSimd tensor_tensor_scan (fp16). The y = sum_n h_n*C_n reduction,
the D*u skip term, and the y_f + y_b combine ride the Tensor engine as
identity/diagonal matmuls accumulating into a PSUM tile (high_priority
so they never queue behind the next combo's conv matmuls). B/C rows
broadcast across partitions via DRAM-source stride-0 DMA, two states
per transfer, with one wide in-place DVE multiply per pair (du repeated
via a stride-0 AP). The per-(dir,chunk) front-end is software-pipelined
one step ahead of the scan phase.

Self-contained: hardcodes all shapes; no sibling imports.
"""
import os
import numpy as np
import ml_dtypes
from contextlib import ExitStack

import concourse.bacc as bacc
import concourse.bass as bass
import concourse.tile as tile
from concourse import mybir
from concourse.bass_utils import run_bass_kernel_spmd

bf = ml_dtypes.bfloat16
f8 = ml_dtypes.float8_e4m3
FP32 = mybir.dt.float32
BF16 = mybir.dt.bfloat16
FP16 = mybir.dt.float16
FP8 = mybir.dt.float8e4

B, L = 2, 4096
LC = 2048
NCH = L // LC
NSUB = LC // 512
N = 16
XPAD = L + 6          # fp8 x tile: 3 left pad + L + 3 right pad
AOP = mybir.AluOpType
AF = mybir.ActivationFunctionType
DR = mybir.MatmulPerfMode.DoubleRow

S_X = 16.0            # host pre-scale on x for fp8
# np_ pairs (0..7) whose dBu and h*C elementwise multiplies run on GpSimd.
# Default OFF: measured on HW, co-running Pool with the DVE slows DVE scans
# ~2.7x (SBUF port contention) and Pool TT itself is ~4x slower than DVE.
POOL_TT = tuple(
    int(t) for t in os.environ.get("BMK_POOL_TT", "").split(",") if t != ""
)


def _bcast_from_dram(nc, dst, row):
    """DMA-broadcast a [1, F] DRAM row across all partitions of dst."""
    rap = [list(x) for x in row.ap]
    src = bass.AP(tensor=row.tensor, offset=row.offset,
                  ap=[[0, dst.shape[0]], rap[1]])
    nc.sync.dma_start(out=dst, in_=src)


def _view3(t2, dim1, dim2):
    """Reshape a 2D slice AP into [part, dim1, dim2] (strides in elems)."""
    return bass.AP(tensor=t2.tensor, offset=t2.offset,
                   ap=[list(t2.ap[0]), list(dim1), list(dim2)])


def build_program(tc, ins, outs):
    nc = tc.nc
    with ExitStack() as ctx:
        wp = ctx.enter_context(tc.tile_pool(name="wp", bufs=1))
        big = ctx.enter_context(tc.tile_pool(name="big", bufs=1))
        work = ctx.enter_context(tc.tile_pool(name="work", bufs=1))
        scanp = ctx.enter_context(tc.tile_pool(name="scanp", bufs=2))
        ps = ctx.enter_context(tc.tile_pool(name="ps", bufs=1, space="PSUM"))
        dramp = ctx.enter_context(tc.tile_pool(name="dramp", bufs=3,
                                               space="DRAM"))

        # ---- weights ----
        # wconst fp32 [128, 76]: An_f 0:16 | An_b 16:32 | (unused) |
        #   convb_f 64:68 | convb_b 68:72 | dtbias_f 74 | dtbias_b 75
        wconst = wp.tile([128, 76], FP32, tag="wconst")
        nc.sync.dma_start(out=wconst, in_=ins["wconst"])
        COL = {"An_f": 0, "An_b": 16, "cb_f": 64, "cb_b": 68,
               "dtb_f": 74, "dtb_b": 75}

        # fp8 conv weights: one [128, 1024] slab per (dir, mt);
        # slab col layout: tap k -> [k*256 : k*256+256] = [kt0 128][kt1 128]
        wconv = wp.tile([128, 8 * 1024], FP8, tag="wconv")
        # load b-dir slabs first: the first combo is b-direction.
        for dcol_mt in ([4 + m for m in range(4)] + list(range(4))):
            nc.sync.dma_start(
                out=wconv[:, dcol_mt * 1024:(dcol_mt + 1) * 1024],
                in_=ins["wconvP8"][:, dcol_mt * 1024:(dcol_mt + 1) * 1024])
        wz = wp.tile([128, 256], FP8, tag="wz")
        outw = wp.tile([128, 256], FP16, tag="outw")
        xpro = wp.tile([128, 384], FP16, tag="xpro")
        dtprojp = wp.tile([16, 256], FP16, tag="dtprojp")
        ident = wp.tile([128, 128], BF16, tag="ident")
        diagD = wp.tile([128, 256], BF16, tag="diagD")
        carry = wp.tile([128, 32], FP32, tag="carry")

        def _late_weight_dmas():
            nc.sync.dma_start(out=wz, in_=ins["wzP8"])
            nc.sync.dma_start(out=outw, in_=ins["outWT"])
            nc.sync.dma_start(out=xpro, in_=ins["xprojP"])
            nc.sync.dma_start(out=dtprojp, in_=ins["dtprojp"])
            nc.sync.dma_start(out=ident, in_=ins["ident"])
            nc.sync.dma_start(out=diagD, in_=ins["diagD"])

        # ---- persistent buffers ----
        # x in fp8: [128, 2*XPAD]; col j*XPAD + 3 + t = x[t] for model dims
        # 128*j + p, pre-scaled by S_X. First combo reads chunk 1 (b-dir),
        # so load the second half of time first.
        x8 = big.tile([128, 2 * XPAD], FP8, tag="x8", name="x8")
        HALF = 3 + LC
        for j in range(2):
            nc.sync.dma_start(
                out=x8[:, j * XPAD + HALF:(j + 1) * XPAD],
                in_=ins["x8p"][:, j * XPAD + HALF:(j + 1) * XPAD])
        _late_weight_dmas()
        for j in range(2):
            nc.sync.dma_start(
                out=x8[:, j * XPAD:j * XPAD + HALF],
                in_=ins["x8p"][:, j * XPAD:j * XPAD + HALF])
        zs_all = big.tile([128, L], FP16, tag="zs")
        y_ball = big.tile([128, L], FP16, tag="yball")

        DSC = {"f": ins["scales"]["f"], "b": ins["scales"]["b"],
               "z": ins["scales"]["z"]}

        def _conv_mm(pt, dcol, mt, ns0):
            """4 fp8 DoubleRow matmuls accumulating conv+in_proj into pt."""
            slab = (dcol * 4 + mt) * 1024
            for k in range(4):
                w2 = wconv[:, slab + k * 256:slab + k * 256 + 256]
                lhsT = _view3(w2, [128, 2], [1, 128])
                xs = x8[:, ns0 + k:ns0 + k + 512]
                rhs = _view3(xs, [XPAD, 2], [1, 512])
                nc.tensor.matmul(pt, lhsT, rhs, start=(k == 0),
                                 stop=(k == 3), perf_mode=DR)

        # ---- phase Z ----
        def phase_z(c):
            for nsub in range(NSUB):
                pt = ps.tile([128, 512], FP32, tag="ps_conv", bufs=2)
                z2 = _view3(wz[:, 0:256], [128, 2], [1, 128])
                xs = x8[:, 3 + c * LC + nsub * 512:3 + c * LC + nsub * 512 + 512]
                rhs = _view3(xs, [XPAD, 2], [1, 512])
                nc.tensor.matmul(pt, z2, rhs, start=True, stop=True,
                                 perf_mode=DR)
                nc.scalar.activation(
                    out=zs_all[:, c * LC + nsub * 512:c * LC + (nsub + 1) * 512],
                    in_=pt, func=AF.Silu, bias=0.0, scale=DSC["z"])

        combos = ([("b", c) for c in range(NCH - 1, -1, -1)]
                  + [("f", c) for c in range(NCH)])

        def front_end(d, c):
            """conv -> Silu(psum*descale+bias) -> xc; x_proj -> dbl; dt.

            nsub-major order with per-half (1024 col) Exp/Ln/du so the dt
            chain is ready half-way into the front-end. For b-dirs the C
            rows are stored time-reversed in dbl (via reversed scalar
            copies) so the scan phase can use one wide h*C multiply.
            """
            dcol = 0 if d == "f" else 1
            base = 0 if d == "f" else 3
            cb0 = COL[f"cb_{d}"]
            xc = [work.tile([128, LC], FP16,
                            tag=(f"xc0{d}" if t == 0 else f"xc{t}"),
                            name=f"xc{t}", bufs=2)
                  for t in range(4)]
            dbl = work.tile([48, LC], FP16, tag="dbl", bufs=2)
            vsub = work.tile([128, LC], FP16, tag="vsub", bufs=2)
            dt = work.tile([128, LC], FP16, tag="dt", bufs=2)
            esub = work.tile([128, LC], FP16, tag="esub", bufs=2)
            du = work.tile([128, LC], FP16, tag="du", bufs=2)
            scratch = dramp.tile([32, LC], FP16, tag="bcdram")

            for nsub in range(NSUB):
                ns0 = c * LC + nsub * 512 + base
                for mt in range(4):
                    pt = ps.tile([128, 512], FP32, tag="ps_conv", bufs=2)
                    _conv_mm(pt, dcol, mt, ns0)
                    nc.scalar.activation(
                        out=xc[mt][:, nsub * 512:(nsub + 1) * 512], in_=pt,
                        func=AF.Silu,
                        bias=wconst[:, cb0 + mt:cb0 + mt + 1],
                        scale=DSC[d])
                pj = ps.tile([48, 512], FP32, tag="ps_small", name="pj", bufs=2)
                for kt in range(4):
                    nc.tensor.matmul(
                        pj, xpro[:, kt * 96 + 48 * dcol:
                                 kt * 96 + 48 * (dcol + 1)],
                        xc[kt][:, nsub * 512:(nsub + 1) * 512],
                        start=(kt == 0), stop=(kt == 3))
                sl = slice(nsub * 512, (nsub + 1) * 512)
                nc.scalar.copy(out=dbl[:, sl], in_=pj)
                ptdt = ps.tile([128, 512], FP32, tag="ps_small", name="ptdt", bufs=2)
                nc.tensor.matmul(
                    ptdt, dtprojp[:, dcol * 128:(dcol + 1) * 128],
                    dbl[0:16, sl], start=True, stop=True)
                # Identity (+dt bias) lives in every act table: no load
                nc.scalar.activation(
                    out=vsub[:, sl], in_=ptdt, func=AF.Identity,
                    bias=wconst[:, COL[f"dtb_{d}"]:COL[f"dtb_{d}"] + 1],
                    scale=1.0)
            nc.sync.dma_start(out=scratch, in_=dbl[16:48, :])

            # softplus = ln(1 + exp(v)) as two whole-chunk ops: exp and ln
            # live in different act tables (native Softplus has none), so
            # batching costs two loads per combo instead of eight.
            nc.scalar.activation(out=esub, in_=vsub, func=AF.Exp,
                                 bias=0.0, scale=1.0)
            nc.scalar.activation(out=dt, in_=esub, func=AF.Ln,
                                 bias=1.0, scale=1.0)
            nc.vector.tensor_tensor(du, dt, xc[0], AOP.mult)
            return {"xc0": xc[0], "dt": dt, "du": du, "scratch": scratch}

        ub_store = {}

        def scan_phase(d, c, st, first):
            dcol = 0 if d == "f" else 1
            rev = (lambda ap: ap[:, ::-1]) if d == "b" else (lambda ap: ap)
            dt, du, scratch, u = st["dt"], st["du"], st["scratch"], st["xc0"]
            if d == "b":
                ub_store[c] = u

            psy = ps.tile([128, LC], FP32, tag="ps_y")

            def bcast_mul(dst2, rows, mul, h0, w):
                """One DMA broadcasting two scratch row-segments [h0:h0+w]
                into both state-halves of dst2, then one in-place multiply
                dst2 *= repeat(mul[:, h0:h0+w], 2)."""
                rs = rows[:, h0:h0 + w]
                rap = [list(x) for x in rs.ap]
                src = bass.AP(tensor=rs.tensor, offset=rs.offset,
                              ap=[[0, 128], rap[0], rap[1]])
                d3 = bass.AP(tensor=dst2.tensor, offset=dst2.offset + h0,
                             ap=[list(dst2.ap[0]), [LC, 2], [1, w]])
                nc.sync.dma_start(out=d3, in_=src)
                mrep = bass.AP(tensor=mul.tensor, offset=mul.offset + h0,
                               ap=[list(mul.ap[0]), [0, 2], [1, w]])
                nc.vector.tensor_tensor(d3, mrep, d3, AOP.mult)

            for np_ in range(N // 2):
                n0 = 2 * np_
                dA2 = scanp.tile([128, 2 * LC], FP16, tag="dA", bufs=2)
                bbc2 = scanp.tile([128, 2 * LC], FP16, tag="bbc", bufs=2)
                h2 = scanp.tile([128, 2 * LC], FP16, tag="h", bufs=2)
                for i in range(2):
                    nc.scalar.activation(
                        out=dA2[:, i * LC:(i + 1) * LC], in_=dt,
                        func=AF.Exp, bias=0.0,
                        scale=wconst[:, COL[f"An_{d}"] + n0 + i:
                                     COL[f"An_{d}"] + n0 + i + 1])
                bcast_mul(bbc2, scratch[n0:n0 + 2, :], du, 0, LC)
                for i in range(2):
                    n = n0 + i
                    hsl = h2[:, i * LC:(i + 1) * LC]
                    init = (0.0 if first
                            else carry[:, dcol * 16 + n:dcol * 16 + n + 1])
                    nc.vector.tensor_tensor_scan(
                        hsl, rev(dA2[:, i * LC:(i + 1) * LC]),
                        rev(bbc2[:, i * LC:(i + 1) * LC]), init,
                        AOP.mult, AOP.add)
                    if first and NCH > 1:
                        nc.vector.tensor_copy(
                            out=carry[:, dcol * 16 + n:dcol * 16 + n + 1],
                            in_=hsl[:, LC - 1:LC])
                cbc2 = scanp.tile([128, 2 * LC], FP16, tag="cbc", bufs=2)
                rap = [list(x) for x in scratch[16 + n0:18 + n0, :].ap]
                src = bass.AP(tensor=scratch.tensor,
                              offset=scratch[16 + n0:18 + n0, :].offset,
                              ap=[[0, 128], rap[0], rap[1]])
                d3 = bass.AP(tensor=cbc2.tensor, offset=cbc2.offset,
                             ap=[list(cbc2.ap[0]), [LC, 2], [1, LC]])
                nc.sync.dma_start(out=d3, in_=src)
                if d == "b":
                    # multiply by reversed-C per half
                    for i in range(2):
                        nc.vector.tensor_tensor(
                            h2[:, i * LC:(i + 1) * LC],
                            h2[:, i * LC:(i + 1) * LC],
                            cbc2[:, i * LC:(i + 1) * LC][:, ::-1], AOP.mult)
                else:
                    h3 = bass.AP(tensor=h2.tensor, offset=h2.offset,
                                 ap=[list(h2.ap[0]), [LC, 2], [1, LC]])
                    c3 = bass.AP(tensor=cbc2.tensor, offset=cbc2.offset,
                                 ap=[list(cbc2.ap[0]), [LC, 2], [1, LC]])
                    nc.vector.tensor_tensor(h3, h3, c3, AOP.mult)
                with tc.high_priority():
                    for i in range(2):
                        n = n0 + i
                        for q in range(NSUB):
                            nc.tensor.matmul(
                                psy[:, q * 512:(q + 1) * 512], ident,
                                h2[:, i * LC + q * 512:i * LC + (q + 1) * 512],
                                start=(n == 0),
                                stop=(d == "b" and n == N - 1),
                                skip_group_check=True)

            if d == "b":
                # The D_b*u_b skip term rides the matching f-combo's PSUM.
                # psy holds y_b in reversed time; un-reverse on copy-out.
                for q in range(NSUB):
                    fseg = 3 - q
                    nc.scalar.copy(
                        out=y_ball[:, c * LC + fseg * 512:
                                   c * LC + (fseg + 1) * 512][:, ::-1],
                        in_=psy[:, q * 512:(q + 1) * 512])
            else:
                u_b = ub_store[c]
                for q in range(NSUB):
                    sl = slice(q * 512, (q + 1) * 512)
                    nc.tensor.matmul(psy[:, sl], diagD[:, 0:128], u[:, sl],
                                     start=False, stop=False,
                                     skip_group_check=True)
                    nc.tensor.matmul(psy[:, sl], diagD[:, 128:256],
                                     u_b[:, sl], start=False, stop=False,
                                     skip_group_check=True)
                    nc.tensor.matmul(
                        psy[:, sl], ident,
                        y_ball[:, c * LC + q * 512:c * LC + (q + 1) * 512],
                        start=False, stop=True, skip_group_check=True)
                ysum = work.tile([128, LC], FP16, tag="ysum", bufs=1)
                ygated = work.tile([128, LC], FP16, tag="ygated", bufs=1)
                with tc.high_priority():
                    for q in range(NSUB):
                        sl = slice(q * 512, (q + 1) * 512)
                        nc.scalar.copy(out=ysum[:, sl], in_=psy[:, sl])
                        nc.vector.tensor_tensor(
                            ygated[:, sl], ysum[:, sl],
                            zs_all[:, c * LC + q * 512:
                                   c * LC + (q + 1) * 512],
                            AOP.mult)
                for mt in range(2):
                    osb = work.tile([128, LC], FP32, tag="osb", bufs=1)
                    for nsub in range(NSUB):
                        po = ps.tile([128, 512], FP32, tag="ps_small", name="po", bufs=2)
                        nc.tensor.matmul(
                            po, outw[:, mt * 128:(mt + 1) * 128],
                            ygated[:, nsub * 512:(nsub + 1) * 512],
                            start=True, stop=True)
                        nc.scalar.copy(
                            out=osb[:, nsub * 512:(nsub + 1) * 512], in_=po)
                        nc.sync.dma_start(
                            out=outs["attnT"][mt * 128:(mt + 1) * 128,
                                              c * LC + nsub * 512:
                                              c * LC + (nsub + 1) * 512],
                            in_=osb[:, nsub * 512:(nsub + 1) * 512])

        # software pipeline: front_end one combo ahead of the scan phase;
        # phase-Z rides in the shadow of the first front-end
        states = {}
        states[0] = front_end(*combos[0])
        for j, (d, c) in enumerate(combos):
            if j + 1 < len(combos):
                # Gate the next combo's front-end behind the fill window so
                # its scalar/PE ops don't wedge into combo j's critical
                # chain on the in-order engines.
                with tc.tile_wait_until(0.045 if j == 0 else 0):
                    states[j + 1] = front_end(*combos[j + 1])
            if j == 1:
                with tc.tile_wait_until(0.110):
                    for c2 in range(NCH):
                        phase_z(c2)
            first = (j % NCH == 0)
            scan_phase(d, c, states.pop(j), first)


def build_nc(scales):
    nc = bacc.Bacc("TRN2", target_bir_lowering=False, debug=False,
                   enable_asserts=False)
    ins = {}

    def inp(name, shape, dt):
        ins[name] = nc.dram_tensor(name, shape, dt,
                                   kind="ExternalInput").ap()

    inp("x8p", [128, 2 * XPAD], FP8)
    inp("wconvP8", [128, 8 * 1024], FP8)
    inp("wzP8", [128, 256], FP8)
    inp("outWT", [128, 256], FP16)
    inp("xprojP", [128, 384], FP16)
    inp("dtprojp", [16, 256], FP16)
    inp("wconst", [128, 76], FP32)
    inp("ident", [128, 128], BF16)
    inp("diagD", [128, 256], BF16)
    ins["scales"] = scales
    outs = {"attnT": nc.dram_tensor("attnT", [256, L], FP32,
                                    kind="ExternalOutput").ap()}
    with tile.TileContext(nc) as tc:
        build_program(tc, ins, outs)
    nc.compile()
    return nc


def _pow2_scale(maxabs, target=192.0):
    """Largest power of 2 s with maxabs*s <= target (e4m3 max 240)."""
    import math
    if maxabs <= 0:
        return 1.0
    return 2.0 ** math.floor(math.log2(target / maxabs))


def prep_scales(inputs):
    """Power-of-2 fp8 pre-scales shared by all cores (weight-dependent)."""
    ipw = inputs["in_proj_w"].astype(np.float64)
    scales = {}
    for d in "fb":
        cw = inputs[f"conv_w_{d}"][:, 0, :].astype(np.float64)
        wmax = (np.abs(cw).max(axis=1)[:, None]
                * np.abs(ipw[:512]).max(axis=1)[:, None]).max()
        # bound on |tap_k * w_inx| entries
        wmax = max((np.abs(cw)[:, :, None]
                    * np.abs(ipw[:512])[:, None, :]).max(), 1e-12)
        scales[d] = _pow2_scale(wmax)
    scales["z"] = _pow2_scale(np.abs(ipw[512:]).max())
    return scales


_CACHE = {}


def prep_core_inputs(inputs, b, dq, scales):
    """Per-core input arrays; d_inner axis permuted so own block is first."""
    own = np.arange(dq * 128, (dq + 1) * 128)
    rest = np.array([i for i in range(512)
                     if not (dq * 128 <= i < (dq + 1) * 128)])
    perm = np.concatenate([own, rest])

    out = {}
    xT = inputs["x"][b].T.astype(np.float32)  # [256, L]
    x8p = np.zeros((128, 2 * XPAD), np.float32)
    for j in range(2):
        x8p[:, j * XPAD + 3:j * XPAD + 3 + L] = xT[j * 128:(j + 1) * 128] * S_X
    out["x8p"] = x8p.astype(f8)

    w_inx = inputs["in_proj_w"][:512][perm].astype(np.float64)  # [512, 256]
    wconvP = np.zeros((128, 8 * 1024), np.float64)
    for dcol, d in enumerate("fb"):
        cw = inputs[f"conv_w_{d}"][:, 0, :][perm].astype(np.float64)
        sw = scales[d]
        for k in range(4):
            tap = cw[:, k] if d == "f" else cw[:, 3 - k]
            WdkT = (tap[:, None] * w_inx).T * sw     # [256, 512]
            for mt in range(4):
                slab = (dcol * 4 + mt) * 1024
                for kt in range(2):
                    off = slab + k * 256 + kt * 128
                    wconvP[:, off:off + 128] = \
                        WdkT[kt * 128:(kt + 1) * 128,
                             mt * 128:(mt + 1) * 128]
    out["wconvP8"] = wconvP.astype(f8)

    wz = inputs["in_proj_w"][512:1024][own].astype(np.float64)  # [128, 256]
    wzP = np.zeros((128, 256), np.float64)
    for kt in range(2):
        wzP[:, kt * 128:(kt + 1) * 128] = wz.T[kt * 128:(kt + 1) * 128]
    out["wzP8"] = (wzP * scales["z"]).astype(f8)

    out["outWT"] = np.ascontiguousarray(
        inputs["out_proj_w"][:, own].T).astype(np.float16)  # [128, 256]

    xprojP = np.zeros((128, 384), np.float32)
    xpf = inputs["xproj_w_f"][:, perm].T  # [512, 48]
    xpb = inputs["xproj_w_b"][:, perm].T
    for kt in range(4):
        xprojP[:, kt * 96:kt * 96 + 48] = xpf[kt * 128:(kt + 1) * 128]
        xprojP[:, kt * 96 + 48:kt * 96 + 96] = xpb[kt * 128:(kt + 1) * 128]
    out["xprojP"] = xprojP.astype(np.float16)

    out["dtprojp"] = np.ascontiguousarray(np.concatenate(
        [inputs["dtproj_w_f"][own].T, inputs["dtproj_w_b"][own].T],
        axis=1)).astype(np.float16)  # [16, 256]

    wconst = np.zeros((128, 76), np.float32)
    for i, d in enumerate("fb"):
        wconst[:, 16 * i:16 * i + 16] = -np.exp(
            inputs[f"A_log_{d}"][own].astype(np.float64))
        cb = inputs[f"conv_b_{d}"][perm]
        wconst[:, 64 + 4 * i:68 + 4 * i] = cb.reshape(4, 128).T
        wconst[:, 74 + i] = inputs[f"dtproj_b_{d}"][own]
    out["wconst"] = wconst

    out["ident"] = np.eye(128, dtype=np.float32).astype(bf)
    diagD = np.zeros((128, 256), np.float32)
    diagD[:, 0:128] = np.diag(inputs["D_f"][own])
    diagD[:, 128:256] = np.diag(inputs["D_b"][own])
    out["diagD"] = diagD.astype(bf)
    return out


def kernel(**inputs):
    inputs = {k: np.asarray(v) for k, v in inputs.items()}
    scales = prep_scales(inputs)
    descales = {"f": 1.0 / (S_X * scales["f"]),
                "b": 1.0 / (S_X * scales["b"]),
                "z": 1.0 / (S_X * scales["z"])}
    if "nc" not in _CACHE:
        _CACHE["nc"] = build_nc(descales)
    nc = _CACHE["nc"]

    core_ids = list(range(8))
    in_maps = [prep_core_inputs(inputs, core // 4, core % 4, scales)
               for core in core_ids]
    trace = os.environ.get("BASS_KERNEL_TRACE", "0") == "1"
    res = run_bass_kernel_spmd(nc, in_maps, core_ids, trace=trace)
    _CACHE["last_results"] = res

    x = inputs["x"].astype(np.float32)
    out = np.empty((B, L, 256), np.float32)
    for b in range(B):
        acc = np.zeros((256, L), np.float32)
        for dq in range(4):
            acc += res.results[4 * b + dq]["attnT"]
        out[b] = x[b] + acc.T
    return out.astype(np.float32)


# revision 26
# speedup vs baseline: 1.0579x; 1.0263x over previous
"""Bidirectional Mamba block (nn_Block_bi_mamba) Trainium2 Bass kernel.

Sharding: 8 cores = (batch b in {0,1}) x (d_inner quarter dq in {0..3}).
Each core computes, for its batch and both scan directions, the full
in_proj+conv (folded into PE matmuls) and x_proj (contracts over all 512
channels), the selective scan for its own 128 channels, and the out_proj
partial product [256, L]. The host sums the 4 partials per batch and
adds the residual x. The d_inner axis is permuted per core so the core's
own channel block is always channel-tile 0, making the device program
identical across cores (SPMD) with only input data differing.

v2 changes vs the 653us baseline:
- The conv-folded in_proj and the z-projection run as fp8e4m3 DoubleRow
  matmuls (256-deep contraction, 0.5 cyc/col): ~4x less PE time. Host
  pre-scales x by 16 and the folded weights by a power-of-2 to fill the
  e4m3 range; the descale rides the (free) activation scale. End-to-end
  error stays ~2e-4 because attn_out is small vs the residual x.
- silu is the native Silu activation applied directly to conv PSUM
  (fused bias + fp8 descale + nonlinearity in one scalar op); this
  drops the sigmoid act table and the DVE silu multiplies. softplus is
  batched per-nsub Identity(+bias) copies (table-load free) followed by
  one whole-chunk Exp and Ln.
- nsub-major front-end: x_proj/dtproj for each 512-col block run right
  after its conv tiles, so the dt chain that gates the next combo's
  first scan completes earlier.
- GpSimd stays idle ON PURPOSE: TensorTensorScan is ISA-illegal on
  Pool, and measured on HW, co-running Pool tensor_tensor with the DVE
  slows DVE scans ~2.7x (SBUF port contention) while Pool itself is
  ~4x slower per element than the DVE's 2x-fp16 mode.

Device layout: d-major [128 chans, time]. The SSM recurrence runs on
the DVE tensor_tensor_scan (fp16, ~2.17ns/elem — the hard floor at
~284us/core). The y = sum_n h_n*C_n reduction,
the D*u skip term, and the y_f + y_b combine ride the Tensor engine as
identity/diagonal matmuls accumulating into a PSUM tile (high_priority
so they never queue behind the next combo's conv matmuls). B/C rows
broadcast across partitions via DRAM-source stride-0 DMA, two states
per transfer, with one wide in-place DVE multiply per pair (du repeated
via a stride-0 AP). The per-(dir,chunk) front-end is software-pipelined
one step ahead of the scan phase.

Self-contained: hardcodes all shapes; no sibling imports.
"""
import os
import numpy as np
import ml_dtypes
from contextlib import ExitStack

import concourse.bacc as bacc
import concourse.bass as bass
import concourse.tile as tile
from concourse import mybir
from concourse.bass_utils import run_bass_kernel_spmd

bf = ml_dtypes.bfloat16
f8 = ml_dtypes.float8_e4m3
FP32 = mybir.dt.float32
BF16 = mybir.dt.bfloat16
FP16 = mybir.dt.float16
FP8 = mybir.dt.float8e4

B, L = 2, 4096
LC = 2048
NCH = L // LC
NSUB = LC // 512
N = 16
XPAD = L + 6          # fp8 x tile: 3 left pad + L + 3 right pad
AOP = mybir.AluOpType
AF = mybir.ActivationFunctionType
DR = mybir.MatmulPerfMode.DoubleRow

S_X = 16.0            # host pre-scale on x for fp8
# np_ pairs (0..7) whose dBu and h*C elementwise multiplies run on GpSimd.
# Default OFF: measured on HW, co-running Pool with the DVE slows DVE scans
# ~2.7x (SBUF port contention) and Pool TT itself is ~4x slower than DVE.
POOL_TT = tuple(
    int(t) for t in os.environ.get("BMK_POOL_TT", "").split(",") if t != ""
)


def _bcast_from_dram(nc, dst, row):
    """DMA-broadcast a [1, F] DRAM row across all partitions of dst."""
    rap = [list(x) for x in row.ap]
    src = bass.AP(tensor=row.tensor, offset=row.offset,
                  ap=[[0, dst.shape[0]], rap[1]])
    nc.sync.dma_start(out=dst, in_=src)


def _view3(t2, dim1, dim2):
    """Reshape a 2D slice AP into [part, dim1, dim2] (strides in elems)."""
    return bass.AP(tensor=t2.tensor, offset=t2.offset,
                   ap=[list(t2.ap[0]), list(dim1), list(dim2)])


def build_program(tc, ins, outs):
    nc = tc.nc
    with ExitStack() as ctx:
        wp = ctx.enter_context(tc.tile_pool(name="wp", bufs=1))
        big = ctx.enter_context(tc.tile_pool(name="big", bufs=1))
        work = ctx.enter_context(tc.tile_pool(name="work", bufs=1))
        scanp = ctx.enter_context(tc.tile_pool(name="scanp", bufs=2))
        ps = ctx.enter_context(tc.tile_pool(name="ps", bufs=1, space="PSUM"))
        dramp = ctx.enter_context(tc.tile_pool(name="dramp", bufs=3,
                                               space="DRAM"))

        # ---- weights ----
        # wconst fp32 [128, 76]: An_f 0:16 | An_b 16:32 | (unused) |
        #   convb_f 64:68 | convb_b 68:72 | dtbias_f 74 | dtbias_b 75
        wconst = wp.tile([128, 76], FP32, tag="wconst")
        nc.sync.dma_start(out=wconst, in_=ins["wconst"])
        COL = {"An_f": 0, "An_b": 16, "cb_f": 64, "cb_b": 68,
               "dtb_f": 74, "dtb_b": 75}

        # fp8 conv weights: one [128, 1024] slab per (dir, mt);
        # slab col layout: tap k -> [k*256 : k*256+256] = [kt0 128][kt1 128]
        wconv = wp.tile([128, 8 * 1024], FP8, tag="wconv")
        # load b-dir slabs first: the first combo is b-direction.
        for dcol_mt in ([4 + m for m in range(4)] + list(range(4))):
            nc.sync.dma_start(
                out=wconv[:, dcol_mt * 1024:(dcol_mt + 1) * 1024],
                in_=ins["wconvP8"][:, dcol_mt * 1024:(dcol_mt + 1) * 1024])
        wz = wp.tile([128, 256], FP8, tag="wz")
        outw = wp.tile([128, 256], FP16, tag="outw")
        xpro = wp.tile([128, 384], FP16, tag="xpro")
        dtprojp = wp.tile([16, 256], FP16, tag="dtprojp")
        ident = wp.tile([128, 128], BF16, tag="ident")
        diagD = wp.tile([128, 256], BF16, tag="diagD")
        carry = wp.tile([128, 32], FP32, tag="carry")

        def _late_weight_dmas():
            nc.sync.dma_start(out=wz, in_=ins["wzP8"])
            nc.sync.dma_start(out=outw, in_=ins["outWT"])
            nc.sync.dma_start(out=xpro, in_=ins["xprojP"])
            nc.sync.dma_start(out=dtprojp, in_=ins["dtprojp"])
            nc.sync.dma_start(out=ident, in_=ins["ident"])
            nc.sync.dma_start(out=diagD, in_=ins["diagD"])

        # ---- persistent buffers ----
        # x in fp8: [128, 2*XPAD]; col j*XPAD + 3 + t = x[t] for model dims
        # 128*j + p, pre-scaled by S_X. First combo reads chunk 1 (b-dir),
        # so load the second half of time first.
        x8 = big.tile([128, 2 * XPAD], FP8, tag="x8", name="x8")
        HALF = 3 + LC
        for j in range(2):
            nc.sync.dma_start(
                out=x8[:, j * XPAD + HALF:(j + 1) * XPAD],
                in_=ins["x8p"][:, j * XPAD + HALF:(j + 1) * XPAD])
        _late_weight_dmas()
        for j in range(2):
            nc.sync.dma_start(
                out=x8[:, j * XPAD:j * XPAD + HALF],
                in_=ins["x8p"][:, j * XPAD:j * XPAD + HALF])
        zs_all = big.tile([128, L], FP16, tag="zs")
        y_ball = big.tile([128, L], FP16, tag="yball")

        DSC = {"f": ins["scales"]["f"], "b": ins["scales"]["b"],
               "z": ins["scales"]["z"]}

        def _conv_mm(pt, dcol, mt, ns0):
            """4 fp8 DoubleRow matmuls accumulating conv+in_proj into pt."""
            slab = (dcol * 4 + mt) * 1024
            for k in range(4):
                w2 = wconv[:, slab + k * 256:slab + k * 256 + 256]
                lhsT = _view3(w2, [128, 2], [1, 128])
                xs = x8[:, ns0 + k:ns0 + k + 512]
                rhs = _view3(xs, [XPAD, 2], [1, 512])
                nc.tensor.matmul(pt, lhsT, rhs, start=(k == 0),
                                 stop=(k == 3), perf_mode=DR)

        # ---- phase Z ----
        def phase_z(c):
            for nsub in range(NSUB):
                pt = ps.tile([128, 512], FP32, tag="ps_conv", bufs=2)
                z2 = _view3(wz[:, 0:256], [128, 2], [1, 128])
                xs = x8[:, 3 + c * LC + nsub * 512:3 + c * LC + nsub * 512 + 512]
                rhs = _view3(xs, [XPAD, 2], [1, 512])
                nc.tensor.matmul(pt, z2, rhs, start=True, stop=True,
                                 perf_mode=DR)
                nc.scalar.activation(
                    out=zs_all[:, c * LC + nsub * 512:c * LC + (nsub + 1) * 512],
                    in_=pt, func=AF.Silu, bias=0.0, scale=DSC["z"])

        combos = ([("b", c) for c in range(NCH - 1, -1, -1)]
                  + [("f", c) for c in range(NCH)])

        def front_end(d, c):
            """conv -> Silu(psum*descale+bias) -> xc; x_proj -> dbl; dt.

            nsub-major order with per-half (1024 col) Exp/Ln/du so the dt
            chain is ready half-way into the front-end. For b-dirs the C
            rows are stored time-reversed in dbl (via reversed scalar
            copies) so the scan phase can use one wide h*C multiply.
            """
            dcol = 0 if d == "f" else 1
            base = 0 if d == "f" else 3
            cb0 = COL[f"cb_{d}"]
            xc = [work.tile([128, LC], FP16,
                            tag=(f"xc0{d}" if t == 0 else f"xc{t}"),
                            name=f"xc{t}", bufs=2)
                  for t in range(4)]
            dbl = work.tile([48, LC], FP16, tag="dbl", bufs=2)
            vsub = work.tile([128, LC], FP16, tag="vsub", bufs=2)
            dt = work.tile([128, LC], FP16, tag="dt", bufs=2)
            esub = work.tile([128, LC], FP16, tag="esub", bufs=2)
            du = work.tile([128, LC], FP16, tag="du", bufs=2)
            scratch = dramp.tile([32, LC], FP16, tag="bcdram")

            for nsub in range(NSUB):
                ns0 = c * LC + nsub * 512 + base
                for mt in range(4):
                    pt = ps.tile([128, 512], FP32, tag="ps_conv", bufs=2)
                    _conv_mm(pt, dcol, mt, ns0)
                    nc.scalar.activation(
                        out=xc[mt][:, nsub * 512:(nsub + 1) * 512], in_=pt,
                        func=AF.Silu,
                        bias=wconst[:, cb0 + mt:cb0 + mt + 1],
                        scale=DSC[d])
                pj = ps.tile([48, 512], FP32, tag="ps_small", name="pj", bufs=2)
                for kt in range(4):
                    nc.tensor.matmul(
                        pj, xpro[:, kt * 96 + 48 * dcol:
                                 kt * 96 + 48 * (dcol + 1)],
                        xc[kt][:, nsub * 512:(nsub + 1) * 512],
                        start=(kt == 0), stop=(kt == 3))
                sl = slice(nsub * 512, (nsub + 1) * 512)
                nc.scalar.copy(out=dbl[:, sl], in_=pj)
                ptdt = ps.tile([128, 512], FP32, tag="ps_small", name="ptdt", bufs=2)
                nc.tensor.matmul(
                    ptdt, dtprojp[:, dcol * 128:(dcol + 1) * 128],
                    dbl[0:16, sl], start=True, stop=True)
                # Identity (+dt bias) lives in every act table: no load
                nc.scalar.activation(
                    out=vsub[:, sl], in_=ptdt, func=AF.Identity,
                    bias=wconst[:, COL[f"dtb_{d}"]:COL[f"dtb_{d}"] + 1],
                    scale=1.0)
            nc.sync.dma_start(out=scratch, in_=dbl[16:48, :])

            # softplus = ln(1 + exp(v)) as two whole-chunk ops: exp and ln
            # live in different act tables (native Softplus has none), so
            # batching costs two loads per combo instead of eight.
            nc.scalar.activation(out=esub, in_=vsub, func=AF.Exp,
                                 bias=0.0, scale=1.0)
            nc.scalar.activation(out=dt, in_=esub, func=AF.Ln,
                                 bias=1.0, scale=1.0)
            nc.vector.tensor_tensor(du, dt, xc[0], AOP.mult)
            return {"xc0": xc[0], "dt": dt, "du": du, "scratch": scratch}

        ub_store = {}

        def scan_phase(d, c, st, first):
            dcol = 0 if d == "f" else 1
            rev = (lambda ap: ap[:, ::-1]) if d == "b" else (lambda ap: ap)
            dt, du, scratch, u = st["dt"], st["du"], st["scratch"], st["xc0"]
            if d == "b":
                ub_store[c] = u

            psy = ps.tile([128, LC], FP32, tag="ps_y")

            def bcast_mul(dst2, rows, mul, h0, w):
                """One DMA broadcasting two scratch row-segments [h0:h0+w]
                into both state-halves of dst2, then one in-place multiply
                dst2 *= repeat(mul[:, h0:h0+w], 2)."""
                rs = rows[:, h0:h0 + w]
                rap = [list(x) for x in rs.ap]
                src = bass.AP(tensor=rs.tensor, offset=rs.offset,
                              ap=[[0, 128], rap[0], rap[1]])
                d3 = bass.AP(tensor=dst2.tensor, offset=dst2.offset + h0,
                             ap=[list(dst2.ap[0]), [LC, 2], [1, w]])
                nc.sync.dma_start(out=d3, in_=src)
                mrep = bass.AP(tensor=mul.tensor, offset=mul.offset + h0,
                               ap=[list(mul.ap[0]), [0, 2], [1, w]])
                nc.vector.tensor_tensor(d3, mrep, d3, AOP.mult)

            for np_ in range(N // 2):
                n0 = 2 * np_
                dA2 = scanp.tile([128, 2 * LC], FP16, tag="dA", bufs=3)
                bbc2 = scanp.tile([128, 2 * LC], FP16, tag="bbc", bufs=2)
                h2 = scanp.tile([128, 2 * LC], FP16, tag="h", bufs=2)
                for i in range(2):
                    nc.scalar.activation(
                        out=dA2[:, i * LC:(i + 1) * LC], in_=dt,
                        func=AF.Exp, bias=0.0,
                        scale=wconst[:, COL[f"An_{d}"] + n0 + i:
                                     COL[f"An_{d}"] + n0 + i + 1])
                bcast_mul(bbc2, scratch[n0:n0 + 2, :], du, 0, LC)
                for i in range(2):
                    n = n0 + i
                    hsl = h2[:, i * LC:(i + 1) * LC]
                    init = (0.0 if first
                            else carry[:, dcol * 16 + n:dcol * 16 + n + 1])
                    nc.vector.tensor_tensor_scan(
                        hsl, rev(dA2[:, i * LC:(i + 1) * LC]),
                        rev(bbc2[:, i * LC:(i + 1) * LC]), init,
                        AOP.mult, AOP.add)
                    if first and NCH > 1:
                        # scalar engine: Copy needs no act table and the
                        # DVE is the bottleneck engine
                        nc.scalar.copy(
                            out=carry[:, dcol * 16 + n:dcol * 16 + n + 1],
                            in_=hsl[:, LC - 1:LC])
                cbc2 = scanp.tile([128, 2 * LC], FP16, tag="cbc", bufs=2)
                rap = [list(x) for x in scratch[16 + n0:18 + n0, :].ap]
                src = bass.AP(tensor=scratch.tensor,
                              offset=scratch[16 + n0:18 + n0, :].offset,
                              ap=[[0, 128], rap[0], rap[1]])
                d3 = bass.AP(tensor=cbc2.tensor, offset=cbc2.offset,
                             ap=[list(cbc2.ap[0]), [LC, 2], [1, LC]])
                nc.sync.dma_start(out=d3, in_=src)
                if d == "b":
                    # multiply by reversed-C per half
                    for i in range(2):
                        nc.vector.tensor_tensor(
                            h2[:, i * LC:(i + 1) * LC],
                            h2[:, i * LC:(i + 1) * LC],
                            cbc2[:, i * LC:(i + 1) * LC][:, ::-1], AOP.mult)
                else:
                    h3 = bass.AP(tensor=h2.tensor, offset=h2.offset,
                                 ap=[list(h2.ap[0]), [LC, 2], [1, LC]])
                    c3 = bass.AP(tensor=cbc2.tensor, offset=cbc2.offset,
                                 ap=[list(cbc2.ap[0]), [LC, 2], [1, LC]])
                    nc.vector.tensor_tensor(h3, h3, c3, AOP.mult)
                with tc.high_priority():
                    for i in range(2):
                        n = n0 + i
                        for q in range(NSUB):
                            nc.tensor.matmul(
                                psy[:, q * 512:(q + 1) * 512], ident,
                                h2[:, i * LC + q * 512:i * LC + (q + 1) * 512],
                                start=(n == 0),
                                stop=(d == "b" and n == N - 1),
                                skip_group_check=True)

            if d == "b":
                # The D_b*u_b skip term rides the matching f-combo's PSUM.
                # psy holds y_b in reversed time; un-reverse on copy-out.
                for q in range(NSUB):
                    fseg = 3 - q
                    nc.scalar.copy(
                        out=y_ball[:, c * LC + fseg * 512:
                                   c * LC + (fseg + 1) * 512][:, ::-1],
                        in_=psy[:, q * 512:(q + 1) * 512])
            else:
                u_b = ub_store[c]
                for q in range(NSUB):
                    sl = slice(q * 512, (q + 1) * 512)
                    nc.tensor.matmul(psy[:, sl], diagD[:, 0:128], u[:, sl],
                                     start=False, stop=False,
                                     skip_group_check=True)
                    nc.tensor.matmul(psy[:, sl], diagD[:, 128:256],
                                     u_b[:, sl], start=False, stop=False,
                                     skip_group_check=True)
                    nc.tensor.matmul(
                        psy[:, sl], ident,
                        y_ball[:, c * LC + q * 512:c * LC + (q + 1) * 512],
                        start=False, stop=True, skip_group_check=True)
                ysum = work.tile([128, LC], FP16, tag="ysum", bufs=1)
                ygated = work.tile([128, LC], FP16, tag="ygated", bufs=1)
                with tc.high_priority():
                    for q in range(NSUB):
                        sl = slice(q * 512, (q + 1) * 512)
                        nc.scalar.copy(out=ysum[:, sl], in_=psy[:, sl])
                        nc.vector.tensor_tensor(
                            ygated[:, sl], ysum[:, sl],
                            zs_all[:, c * LC + q * 512:
                                   c * LC + (q + 1) * 512],
                            AOP.mult)
                for mt in range(2):
                    osb = work.tile([128, LC], FP32, tag="osb", bufs=1)
                    for nsub in range(NSUB):
                        po = ps.tile([128, 512], FP32, tag="ps_small", name="po", bufs=2)
                        nc.tensor.matmul(
                            po, outw[:, mt * 128:(mt + 1) * 128],
                            ygated[:, nsub * 512:(nsub + 1) * 512],
                            start=True, stop=True)
                        nc.scalar.copy(
                            out=osb[:, nsub * 512:(nsub + 1) * 512], in_=po)
                        nc.sync.dma_start(
                            out=outs["attnT"][mt * 128:(mt + 1) * 128,
                                              c * LC + nsub * 512:
                                              c * LC + (nsub + 1) * 512],
                            in_=osb[:, nsub * 512:(nsub + 1) * 512])

        # software pipeline: front_end one combo ahead of the scan phase;
        # phase-Z rides in the shadow of the first front-end
        states = {}
        states[0] = front_end(*combos[0])
        for j, (d, c) in enumerate(combos):
            if j + 1 < len(combos):
                # Gate the next combo's front-end behind the fill window so
                # its scalar/PE ops don't wedge into combo j's critical
                # chain on the in-order engines.
                with tc.tile_wait_until(0.045 if j == 0 else 0):
                    states[j + 1] = front_end(*combos[j + 1])
            if j == 1:
                with tc.tile_wait_until(0.110):
                    for c2 in range(NCH):
                        phase_z(c2)
            first = (j % NCH == 0)
            scan_phase(d, c, states.pop(j), first)


def build_nc(scales):
    nc = bacc.Bacc("TRN2", target_bir_lowering=False, debug=False,
                   enable_asserts=False)
    ins = {}

    def inp(name, shape, dt):
        ins[name] = nc.dram_tensor(name, shape, dt,
                                   kind="ExternalInput").ap()

    inp("x8p", [128, 2 * XPAD], FP8)
    inp("wconvP8", [128, 8 * 1024], FP8)
    inp("wzP8", [128, 256], FP8)
    inp("outWT", [128, 256], FP16)
    inp("xprojP", [128, 384], FP16)
    inp("dtprojp", [16, 256], FP16)
    inp("wconst", [128, 76], FP32)
    inp("ident", [128, 128], BF16)
    inp("diagD", [128, 256], BF16)
    ins["scales"] = scales
    outs = {"attnT": nc.dram_tensor("attnT", [256, L], FP32,
                                    kind="ExternalOutput").ap()}
    with tile.TileContext(nc) as tc:
        build_program(tc, ins, outs)
    nc.compile()
    return nc


def _pow2_scale(maxabs, target=192.0):
    """Largest power of 2 s with maxabs*s <= target (e4m3 max 240)."""
    import math
    if maxabs <= 0:
        return 1.0
    return 2.0 ** math.floor(math.log2(target / maxabs))


def prep_scales(inputs):
    """Power-of-2 fp8 pre-scales shared by all cores (weight-dependent)."""
    ipw = inputs["in_proj_w"].astype(np.float64)
    scales = {}
    for d in "fb":
        cw = inputs[f"conv_w_{d}"][:, 0, :].astype(np.float64)
        wmax = (np.abs(cw).max(axis=1)[:, None]
                * np.abs(ipw[:512]).max(axis=1)[:, None]).max()
        # bound on |tap_k * w_inx| entries
        wmax = max((np.abs(cw)[:, :, None]
                    * np.abs(ipw[:512])[:, None, :]).max(), 1e-12)
        scales[d] = _pow2_scale(wmax)
    scales["z"] = _pow2_scale(np.abs(ipw[512:]).max())
    return scales


_CACHE = {}


def prep_core_inputs(inputs, b, dq, scales):
    """Per-core input arrays; d_inner axis permuted so own block is first."""
    own = np.arange(dq * 128, (dq + 1) * 128)
    rest = np.array([i for i in range(512)
                     if not (dq * 128 <= i < (dq + 1) * 128)])
    perm = np.concatenate([own, rest])

    out = {}
    xT = inputs["x"][b].T.astype(np.float32)  # [256, L]
    x8p = np.zeros((128, 2 * XPAD), np.float32)
    for j in range(2):
        x8p[:, j * XPAD + 3:j * XPAD + 3 + L] = xT[j * 128:(j + 1) * 128] * S_X
    out["x8p"] = x8p.astype(f8)

    w_inx = inputs["in_proj_w"][:512][perm].astype(np.float64)  # [512, 256]
    wconvP = np.zeros((128, 8 * 1024), np.float64)
    for dcol, d in enumerate("fb"):
        cw = inputs[f"conv_w_{d}"][:, 0, :][perm].astype(np.float64)
        sw = scales[d]
        for k in range(4):
            tap = cw[:, k] if d == "f" else cw[:, 3 - k]
            WdkT = (tap[:, None] * w_inx).T * sw     # [256, 512]
            for mt in range(4):
                slab = (dcol * 4 + mt) * 1024
                for kt in range(2):
                    off = slab + k * 256 + kt * 128
                    wconvP[:, off:off + 128] = \
                        WdkT[kt * 128:(kt + 1) * 128,
                             mt * 128:(mt + 1) * 128]
    out["wconvP8"] = wconvP.astype(f8)

    wz = inputs["in_proj_w"][512:1024][own].astype(np.float64)  # [128, 256]
    wzP = np.zeros((128, 256), np.float64)
    for kt in range(2):
        wzP[:, kt * 128:(kt + 1) * 128] = wz.T[kt * 128:(kt + 1) * 128]
    out["wzP8"] = (wzP * scales["z"]).astype(f8)

    out["outWT"] = np.ascontiguousarray(
        inputs["out_proj_w"][:, own].T).astype(np.float16)  # [128, 256]

    xprojP = np.zeros((128, 384), np.float32)
    xpf = inputs["xproj_w_f"][:, perm].T  # [512, 48]
    xpb = inputs["xproj_w_b"][:, perm].T
    for kt in range(4):
        xprojP[:, kt * 96:kt * 96 + 48] = xpf[kt * 128:(kt + 1) * 128]
        xprojP[:, kt * 96 + 48:kt * 96 + 96] = xpb[kt * 128:(kt + 1) * 128]
    out["xprojP"] = xprojP.astype(np.float16)

    out["dtprojp"] = np.ascontiguousarray(np.concatenate(
        [inputs["dtproj_w_f"][own].T, inputs["dtproj_w_b"][own].T],
        axis=1)).astype(np.float16)  # [16, 256]

    wconst = np.zeros((128, 76), np.float32)
    for i, d in enumerate("fb"):
        wconst[:, 16 * i:16 * i + 16] = -np.exp(
            inputs[f"A_log_{d}"][own].astype(np.float64))
        cb = inputs[f"conv_b_{d}"][perm]
        wconst[:, 64 + 4 * i:68 + 4 * i] = cb.reshape(4, 128).T
        wconst[:, 74 + i] = inputs[f"dtproj_b_{d}"][own]
    out["wconst"] = wconst

    out["ident"] = np.eye(128, dtype=np.float32).astype(bf)
    diagD = np.zeros((128, 256), np.float32)
    diagD[:, 0:128] = np.diag(inputs["D_f"][own])
    diagD[:, 128:256] = np.diag(inputs["D_b"][own])
    out["diagD"] = diagD.astype(bf)
    return out


def kernel(**inputs):
    inputs = {k: np.asarray(v) for k, v in inputs.items()}
    scales = prep_scales(inputs)
    descales = {"f": 1.0 / (S_X * scales["f"]),
                "b": 1.0 / (S_X * scales["b"]),
                "z": 1.0 / (S_X * scales["z"])}
    if "nc" not in _CACHE:
        _CACHE["nc"] = build_nc(descales)
    nc = _CACHE["nc"]

    core_ids = list(range(8))
    in_maps = [prep_core_inputs(inputs, core // 4, core % 4, scales)
               for core in core_ids]
    trace = os.environ.get("BASS_KERNEL_TRACE", "0") == "1"
    res = run_bass_kernel_spmd(nc, in_maps, core_ids, trace=trace)
    _CACHE["last_results"] = res

    x = inputs["x"].astype(np.float32)
    out = np.empty((B, L, 256), np.float32)
    for b in range(B):
        acc = np.zeros((256, L), np.float32)
        for dq in range(4):
            acc += res.results[4 * b + dq]["attnT"]
        out[b] = x[b] + acc.T
    return out.astype(np.float32)


# revision 27
# speedup vs baseline: 1.0691x; 1.0105x over previous
"""Bidirectional Mamba block (nn_Block_bi_mamba) Trainium2 Bass kernel.

Sharding: 8 cores = (batch b in {0,1}) x (d_inner quarter dq in {0..3}).
Each core computes, for its batch and both scan directions, the full
in_proj+conv (folded into PE matmuls) and x_proj (contracts over all 512
channels), the selective scan for its own 128 channels, and the out_proj
partial product [256, L]. The host sums the 4 partials per batch and
adds the residual x. The d_inner axis is permuted per core so the core's
own channel block is always channel-tile 0, making the device program
identical across cores (SPMD) with only input data differing.

v2 changes vs the 653us baseline:
- The conv-folded in_proj and the z-projection run as fp8e4m3 DoubleRow
  matmuls (256-deep contraction, 0.5 cyc/col): ~4x less PE time. Host
  pre-scales x by 16 and the folded weights by a power-of-2 to fill the
  e4m3 range; the descale rides the (free) activation scale. End-to-end
  error stays ~2e-4 because attn_out is small vs the residual x.
- silu is the native Silu activation applied directly to conv PSUM
  (fused bias + fp8 descale + nonlinearity in one scalar op); this
  drops the sigmoid act table and the DVE silu multiplies. softplus is
  batched per-nsub Identity(+bias) copies (table-load free) followed by
  one whole-chunk Exp and Ln.
- nsub-major front-end: x_proj/dtproj for each 512-col block run right
  after its conv tiles, so the dt chain that gates the next combo's
  first scan completes earlier.
- GpSimd stays idle ON PURPOSE: TensorTensorScan is ISA-illegal on
  Pool, and measured on HW, co-running Pool tensor_tensor with the DVE
  slows DVE scans ~2.7x (SBUF port contention) while Pool itself is
  ~4x slower per element than the DVE's 2x-fp16 mode.

Device layout: d-major [128 chans, time]. The SSM recurrence runs on
the DVE tensor_tensor_scan (fp16, ~2.17ns/elem — the hard floor at
~284us/core). The y = sum_n h_n*C_n reduction,
the D*u skip term, and the y_f + y_b combine ride the Tensor engine as
identity/diagonal matmuls accumulating into a PSUM tile (high_priority
so they never queue behind the next combo's conv matmuls). B/C rows
broadcast across partitions via DRAM-source stride-0 DMA, two states
per transfer, with one wide in-place DVE multiply per pair (du repeated
via a stride-0 AP). The per-(dir,chunk) front-end is software-pipelined
one step ahead of the scan phase.

Self-contained: hardcodes all shapes; no sibling imports.
"""
import os
import numpy as np
import ml_dtypes
from contextlib import ExitStack

import concourse.bacc as bacc
import concourse.bass as bass
import concourse.tile as tile
from concourse import mybir
from concourse.bass_utils import run_bass_kernel_spmd

bf = ml_dtypes.bfloat16
f8 = ml_dtypes.float8_e4m3
FP32 = mybir.dt.float32
BF16 = mybir.dt.bfloat16
FP16 = mybir.dt.float16
FP8 = mybir.dt.float8e4

B, L = 2, 4096
LC = 2048
NCH = L // LC
NSUB = LC // 512
N = 16
XPAD = L + 6          # fp8 x tile: 3 left pad + L + 3 right pad
AOP = mybir.AluOpType
AF = mybir.ActivationFunctionType
DR = mybir.MatmulPerfMode.DoubleRow

S_X = 16.0            # host pre-scale on x for fp8
# np_ pairs (0..7) whose dBu and h*C elementwise multiplies run on GpSimd.
# Default OFF: measured on HW, co-running Pool with the DVE slows DVE scans
# ~2.7x (SBUF port contention) and Pool TT itself is ~4x slower than DVE.
POOL_TT = tuple(
    int(t) for t in os.environ.get("BMK_POOL_TT", "").split(",") if t != ""
)


def _bcast_from_dram(nc, dst, row):
    """DMA-broadcast a [1, F] DRAM row across all partitions of dst."""
    rap = [list(x) for x in row.ap]
    src = bass.AP(tensor=row.tensor, offset=row.offset,
                  ap=[[0, dst.shape[0]], rap[1]])
    nc.sync.dma_start(out=dst, in_=src)


def _view3(t2, dim1, dim2):
    """Reshape a 2D slice AP into [part, dim1, dim2] (strides in elems)."""
    return bass.AP(tensor=t2.tensor, offset=t2.offset,
                   ap=[list(t2.ap[0]), list(dim1), list(dim2)])


def build_program(tc, ins, outs):
    nc = tc.nc
    with ExitStack() as ctx:
        wp = ctx.enter_context(tc.tile_pool(name="wp", bufs=1))
        big = ctx.enter_context(tc.tile_pool(name="big", bufs=1))
        work = ctx.enter_context(tc.tile_pool(name="work", bufs=1))
        scanp = ctx.enter_context(tc.tile_pool(name="scanp", bufs=2))
        ps = ctx.enter_context(tc.tile_pool(name="ps", bufs=1, space="PSUM"))
        dramp = ctx.enter_context(tc.tile_pool(name="dramp", bufs=3,
                                               space="DRAM"))

        # ---- weights ----
        # wconst fp32 [128, 76]: An_f 0:16 | An_b 16:32 | (unused) |
        #   convb_f 64:68 | convb_b 68:72 | dtbias_f 74 | dtbias_b 75
        wconst = wp.tile([128, 76], FP32, tag="wconst")
        nc.sync.dma_start(out=wconst, in_=ins["wconst"])
        COL = {"An_f": 0, "An_b": 16, "cb_f": 64, "cb_b": 68,
               "dtb_f": 74, "dtb_b": 75}

        # fp8 conv weights: one [128, 1024] slab per (dir, mt);
        # slab col layout: tap k -> [k*256 : k*256+256] = [kt0 128][kt1 128]
        wconv = wp.tile([128, 8 * 1024], FP8, tag="wconv")
        # load b-dir slabs first: the first combo is b-direction.
        for dcol_mt in ([4 + m for m in range(4)] + list(range(4))):
            nc.sync.dma_start(
                out=wconv[:, dcol_mt * 1024:(dcol_mt + 1) * 1024],
                in_=ins["wconvP8"][:, dcol_mt * 1024:(dcol_mt + 1) * 1024])
        wz = wp.tile([128, 256], FP8, tag="wz")
        outw = wp.tile([128, 256], FP16, tag="outw")
        xpro = wp.tile([128, 384], FP16, tag="xpro")
        dtprojp = wp.tile([16, 256], FP16, tag="dtprojp")
        ident = wp.tile([128, 128], BF16, tag="ident")
        diagD = wp.tile([128, 256], BF16, tag="diagD")
        carry = wp.tile([128, 32], FP32, tag="carry")

        def _late_weight_dmas():
            nc.sync.dma_start(out=wz, in_=ins["wzP8"])
            nc.sync.dma_start(out=outw, in_=ins["outWT"])
            nc.sync.dma_start(out=xpro, in_=ins["xprojP"])
            nc.sync.dma_start(out=dtprojp, in_=ins["dtprojp"])
            nc.sync.dma_start(out=ident, in_=ins["ident"])
            nc.sync.dma_start(out=diagD, in_=ins["diagD"])

        # ---- persistent buffers ----
        # x in fp8: [128, 2*XPAD]; col j*XPAD + 3 + t = x[t] for model dims
        # 128*j + p, pre-scaled by S_X. First combo reads chunk 1 (b-dir),
        # so load the second half of time first.
        x8 = big.tile([128, 2 * XPAD], FP8, tag="x8", name="x8")
        HALF = 3 + LC
        for j in range(2):
            nc.sync.dma_start(
                out=x8[:, j * XPAD + HALF:(j + 1) * XPAD],
                in_=ins["x8p"][:, j * XPAD + HALF:(j + 1) * XPAD])
        _late_weight_dmas()
        for j in range(2):
            nc.sync.dma_start(
                out=x8[:, j * XPAD:j * XPAD + HALF],
                in_=ins["x8p"][:, j * XPAD:j * XPAD + HALF])
        zs_all = big.tile([128, L], FP16, tag="zs")
        y_ball = big.tile([128, L], FP16, tag="yball")

        DSC = {"f": ins["scales"]["f"], "b": ins["scales"]["b"],
               "z": ins["scales"]["z"]}

        def _conv_mm(pt, dcol, mt, ns0):
            """4 fp8 DoubleRow matmuls accumulating conv+in_proj into pt."""
            slab = (dcol * 4 + mt) * 1024
            for k in range(4):
                w2 = wconv[:, slab + k * 256:slab + k * 256 + 256]
                lhsT = _view3(w2, [128, 2], [1, 128])
                xs = x8[:, ns0 + k:ns0 + k + 512]
                rhs = _view3(xs, [XPAD, 2], [1, 512])
                nc.tensor.matmul(pt, lhsT, rhs, start=(k == 0),
                                 stop=(k == 3), perf_mode=DR)

        # ---- phase Z ----
        def phase_z(c):
            for nsub in range(NSUB):
                pt = ps.tile([128, 512], FP32, tag="ps_conv", bufs=2)
                z2 = _view3(wz[:, 0:256], [128, 2], [1, 128])
                xs = x8[:, 3 + c * LC + nsub * 512:3 + c * LC + nsub * 512 + 512]
                rhs = _view3(xs, [XPAD, 2], [1, 512])
                nc.tensor.matmul(pt, z2, rhs, start=True, stop=True,
                                 perf_mode=DR)
                nc.scalar.activation(
                    out=zs_all[:, c * LC + nsub * 512:c * LC + (nsub + 1) * 512],
                    in_=pt, func=AF.Silu, bias=0.0, scale=DSC["z"])

        combos = ([("b", c) for c in range(NCH - 1, -1, -1)]
                  + [("f", c) for c in range(NCH)])

        def front_end(d, c):
            """conv -> Silu(psum*descale+bias) -> xc; x_proj -> dbl; dt.

            nsub-major order with per-half (1024 col) Exp/Ln/du so the dt
            chain is ready half-way into the front-end. For b-dirs the C
            rows are stored time-reversed in dbl (via reversed scalar
            copies) so the scan phase can use one wide h*C multiply.
            """
            dcol = 0 if d == "f" else 1
            base = 0 if d == "f" else 3
            cb0 = COL[f"cb_{d}"]
            xc = [work.tile([128, LC], FP16,
                            tag=(f"xc0{d}" if t == 0 else f"xc{t}"),
                            name=f"xc{t}", bufs=2)
                  for t in range(4)]
            dbl = work.tile([48, LC], FP16, tag="dbl", bufs=2)
            vsub = work.tile([128, LC], FP16, tag="vsub", bufs=2)
            dt = work.tile([128, LC], FP16, tag="dt", bufs=2)
            esub = work.tile([128, LC], FP16, tag="esub", bufs=2)
            du = work.tile([128, LC], FP16, tag="du", bufs=2)
            scratch = dramp.tile([32, LC], FP16, tag="bcdram")

            for nsub in range(NSUB):
                ns0 = c * LC + nsub * 512 + base
                for mt in range(4):
                    pt = ps.tile([128, 512], FP32, tag="ps_conv", bufs=2)
                    _conv_mm(pt, dcol, mt, ns0)
                    nc.scalar.activation(
                        out=xc[mt][:, nsub * 512:(nsub + 1) * 512], in_=pt,
                        func=AF.Silu,
                        bias=wconst[:, cb0 + mt:cb0 + mt + 1],
                        scale=DSC[d])
                pj = ps.tile([48, 512], FP32, tag="ps_small", name="pj", bufs=2)
                for kt in range(4):
                    nc.tensor.matmul(
                        pj, xpro[:, kt * 96 + 48 * dcol:
                                 kt * 96 + 48 * (dcol + 1)],
                        xc[kt][:, nsub * 512:(nsub + 1) * 512],
                        start=(kt == 0), stop=(kt == 3))
                sl = slice(nsub * 512, (nsub + 1) * 512)
                nc.scalar.copy(out=dbl[:, sl], in_=pj)
                ptdt = ps.tile([128, 512], FP32, tag="ps_small", name="ptdt", bufs=2)
                nc.tensor.matmul(
                    ptdt, dtprojp[:, dcol * 128:(dcol + 1) * 128],
                    dbl[0:16, sl], start=True, stop=True)
                # Identity (+dt bias) lives in every act table: no load
                nc.scalar.activation(
                    out=vsub[:, sl], in_=ptdt, func=AF.Identity,
                    bias=wconst[:, COL[f"dtb_{d}"]:COL[f"dtb_{d}"] + 1],
                    scale=1.0)
            nc.sync.dma_start(out=scratch, in_=dbl[16:48, :])

            # softplus = ln(1 + exp(v)) as two whole-chunk ops: exp and ln
            # live in different act tables (native Softplus has none), so
            # batching costs two loads per combo instead of eight.
            nc.scalar.activation(out=esub, in_=vsub, func=AF.Exp,
                                 bias=0.0, scale=1.0)
            nc.scalar.activation(out=dt, in_=esub, func=AF.Ln,
                                 bias=1.0, scale=1.0)
            nc.vector.tensor_tensor(du, dt, xc[0], AOP.mult)
            return {"xc0": xc[0], "dt": dt, "du": du, "scratch": scratch}

        ub_store = {}

        def scan_phase(d, c, st, first):
            dcol = 0 if d == "f" else 1
            rev = (lambda ap: ap[:, ::-1]) if d == "b" else (lambda ap: ap)
            dt, du, scratch, u = st["dt"], st["du"], st["scratch"], st["xc0"]
            if d == "b":
                ub_store[c] = u

            psy = ps.tile([128, LC], FP32, tag="ps_y")

            def bcast_mul(dst2, rows, mul, h0, w):
                """One DMA broadcasting two scratch row-segments [h0:h0+w]
                into both state-halves of dst2, then one in-place multiply
                dst2 *= repeat(mul[:, h0:h0+w], 2)."""
                rs = rows[:, h0:h0 + w]
                rap = [list(x) for x in rs.ap]
                src = bass.AP(tensor=rs.tensor, offset=rs.offset,
                              ap=[[0, 128], rap[0], rap[1]])
                d3 = bass.AP(tensor=dst2.tensor, offset=dst2.offset + h0,
                             ap=[list(dst2.ap[0]), [LC, 2], [1, w]])
                nc.sync.dma_start(out=d3, in_=src)
                mrep = bass.AP(tensor=mul.tensor, offset=mul.offset + h0,
                               ap=[list(mul.ap[0]), [0, 2], [1, w]])
                nc.vector.tensor_tensor(d3, mrep, d3, AOP.mult)

            for np_ in range(N // 2):
                n0 = 2 * np_
                dA2 = scanp.tile([128, 2 * LC], FP16, tag="dA", bufs=3)
                bbc2 = scanp.tile([128, 2 * LC], FP16, tag="bbc", bufs=2)
                h2 = scanp.tile([128, 2 * LC], FP16, tag="h", bufs=2)
                for i in range(2):
                    nc.scalar.activation(
                        out=dA2[:, i * LC:(i + 1) * LC], in_=dt,
                        func=AF.Exp, bias=0.0,
                        scale=wconst[:, COL[f"An_{d}"] + n0 + i:
                                     COL[f"An_{d}"] + n0 + i + 1])
                bcast_mul(bbc2, scratch[n0:n0 + 2, :], du, 0, LC)
                for i in range(2):
                    n = n0 + i
                    hsl = h2[:, i * LC:(i + 1) * LC]
                    init = (0.0 if first
                            else carry[:, dcol * 16 + n:dcol * 16 + n + 1])
                    nc.vector.tensor_tensor_scan(
                        hsl, rev(dA2[:, i * LC:(i + 1) * LC]),
                        rev(bbc2[:, i * LC:(i + 1) * LC]), init,
                        AOP.mult, AOP.add)
                    if first and NCH > 1:
                        # scalar engine: Copy needs no act table and the
                        # DVE is the bottleneck engine
                        nc.scalar.copy(
                            out=carry[:, dcol * 16 + n:dcol * 16 + n + 1],
                            in_=hsl[:, LC - 1:LC])
                cbc2 = scanp.tile([128, 2 * LC], FP16, tag="cbc", bufs=2)
                rap = [list(x) for x in scratch[16 + n0:18 + n0, :].ap]
                src = bass.AP(tensor=scratch.tensor,
                              offset=scratch[16 + n0:18 + n0, :].offset,
                              ap=[[0, 128], rap[0], rap[1]])
                d3 = bass.AP(tensor=cbc2.tensor, offset=cbc2.offset,
                             ap=[list(cbc2.ap[0]), [LC, 2], [1, LC]])
                nc.sync.dma_start(out=d3, in_=src)
                if d == "b":
                    # multiply by reversed-C per half
                    for i in range(2):
                        nc.vector.tensor_tensor(
                            h2[:, i * LC:(i + 1) * LC],
                            h2[:, i * LC:(i + 1) * LC],
                            cbc2[:, i * LC:(i + 1) * LC][:, ::-1], AOP.mult)
                else:
                    h3 = bass.AP(tensor=h2.tensor, offset=h2.offset,
                                 ap=[list(h2.ap[0]), [LC, 2], [1, LC]])
                    c3 = bass.AP(tensor=cbc2.tensor, offset=cbc2.offset,
                                 ap=[list(cbc2.ap[0]), [LC, 2], [1, LC]])
                    nc.vector.tensor_tensor(h3, h3, c3, AOP.mult)
                with tc.high_priority():
                    for i in range(2):
                        n = n0 + i
                        for q in range(NSUB):
                            nc.tensor.matmul(
                                psy[:, q * 512:(q + 1) * 512], ident,
                                h2[:, i * LC + q * 512:i * LC + (q + 1) * 512],
                                start=(n == 0),
                                stop=(d == "b" and n == N - 1),
                                skip_group_check=True)

            if d == "b":
                # The D_b*u_b skip term rides the matching f-combo's PSUM.
                # psy holds y_b in reversed time; un-reverse on copy-out.
                for q in range(NSUB):
                    fseg = 3 - q
                    nc.scalar.copy(
                        out=y_ball[:, c * LC + fseg * 512:
                                   c * LC + (fseg + 1) * 512][:, ::-1],
                        in_=psy[:, q * 512:(q + 1) * 512])
            else:
                u_b = ub_store[c]
                for q in range(NSUB):
                    sl = slice(q * 512, (q + 1) * 512)
                    nc.tensor.matmul(psy[:, sl], diagD[:, 0:128], u[:, sl],
                                     start=False, stop=False,
                                     skip_group_check=True)
                    nc.tensor.matmul(psy[:, sl], diagD[:, 128:256],
                                     u_b[:, sl], start=False, stop=False,
                                     skip_group_check=True)
                    nc.tensor.matmul(
                        psy[:, sl], ident,
                        y_ball[:, c * LC + q * 512:c * LC + (q + 1) * 512],
                        start=False, stop=True, skip_group_check=True)
                ysum = work.tile([128, LC], FP16, tag="ysum", bufs=1)
                ygated = work.tile([128, LC], FP16, tag="ygated", bufs=1)
                with tc.high_priority():
                    for q in range(NSUB):
                        sl = slice(q * 512, (q + 1) * 512)
                        nc.scalar.copy(out=ysum[:, sl], in_=psy[:, sl])
                        nc.vector.tensor_tensor(
                            ygated[:, sl], ysum[:, sl],
                            zs_all[:, c * LC + q * 512:
                                   c * LC + (q + 1) * 512],
                            AOP.mult)
                for mt in range(2):
                    for nsub in range(NSUB):
                        po = ps.tile([128, 512], FP32, tag="ps_small", name="po", bufs=2)
                        nc.tensor.matmul(
                            po, outw[:, mt * 128:(mt + 1) * 128],
                            ygated[:, nsub * 512:(nsub + 1) * 512],
                            start=True, stop=True)
                        # 4 rotating 512-col buffers (same footprint as one
                        # [128, LC] tile): mt1's copies no longer serialize
                        # behind mt0's output DMA
                        osb = work.tile([128, 512], FP32, tag="osb", bufs=4)
                        nc.scalar.copy(out=osb, in_=po)
                        nc.sync.dma_start(
                            out=outs["attnT"][mt * 128:(mt + 1) * 128,
                                              c * LC + nsub * 512:
                                              c * LC + (nsub + 1) * 512],
                            in_=osb)

        # software pipeline: front_end one combo ahead of the scan phase;
        # phase-Z rides in the shadow of the first front-end
        states = {}
        states[0] = front_end(*combos[0])
        for j, (d, c) in enumerate(combos):
            if j + 1 < len(combos):
                # Gate the next combo's front-end behind the fill window so
                # its scalar/PE ops don't wedge into combo j's critical
                # chain on the in-order engines.
                with tc.tile_wait_until(0.045 if j == 0 else 0):
                    states[j + 1] = front_end(*combos[j + 1])
            if j == 1:
                with tc.tile_wait_until(0.150):
                    for c2 in range(NCH):
                        phase_z(c2)
            first = (j % NCH == 0)
            scan_phase(d, c, states.pop(j), first)


def build_nc(scales):
    nc = bacc.Bacc("TRN2", target_bir_lowering=False, debug=False,
                   enable_asserts=False)
    ins = {}

    def inp(name, shape, dt):
        ins[name] = nc.dram_tensor(name, shape, dt,
                                   kind="ExternalInput").ap()

    inp("x8p", [128, 2 * XPAD], FP8)
    inp("wconvP8", [128, 8 * 1024], FP8)
    inp("wzP8", [128, 256], FP8)
    inp("outWT", [128, 256], FP16)
    inp("xprojP", [128, 384], FP16)
    inp("dtprojp", [16, 256], FP16)
    inp("wconst", [128, 76], FP32)
    inp("ident", [128, 128], BF16)
    inp("diagD", [128, 256], BF16)
    ins["scales"] = scales
    outs = {"attnT": nc.dram_tensor("attnT", [256, L], FP32,
                                    kind="ExternalOutput").ap()}
    with tile.TileContext(nc) as tc:
        build_program(tc, ins, outs)
    nc.compile()
    return nc


def _pow2_scale(maxabs, target=192.0):
    """Largest power of 2 s with maxabs*s <= target (e4m3 max 240)."""
    import math
    if maxabs <= 0:
        return 1.0
    return 2.0 ** math.floor(math.log2(target / maxabs))


def prep_scales(inputs):
    """Power-of-2 fp8 pre-scales shared by all cores (weight-dependent)."""
    ipw = inputs["in_proj_w"].astype(np.float64)
    scales = {}
    for d in "fb":
        cw = inputs[f"conv_w_{d}"][:, 0, :].astype(np.float64)
        wmax = (np.abs(cw).max(axis=1)[:, None]
                * np.abs(ipw[:512]).max(axis=1)[:, None]).max()
        # bound on |tap_k * w_inx| entries
        wmax = max((np.abs(cw)[:, :, None]
                    * np.abs(ipw[:512])[:, None, :]).max(), 1e-12)
        scales[d] = _pow2_scale(wmax)
    scales["z"] = _pow2_scale(np.abs(ipw[512:]).max())
    return scales


_CACHE = {}


def prep_core_inputs(inputs, b, dq, scales):
    """Per-core input arrays; d_inner axis permuted so own block is first."""
    own = np.arange(dq * 128, (dq + 1) * 128)
    rest = np.array([i for i in range(512)
                     if not (dq * 128 <= i < (dq + 1) * 128)])
    perm = np.concatenate([own, rest])

    out = {}
    xT = inputs["x"][b].T.astype(np.float32)  # [256, L]
    x8p = np.zeros((128, 2 * XPAD), np.float32)
    for j in range(2):
        x8p[:, j * XPAD + 3:j * XPAD + 3 + L] = xT[j * 128:(j + 1) * 128] * S_X
    out["x8p"] = x8p.astype(f8)

    w_inx = inputs["in_proj_w"][:512][perm].astype(np.float64)  # [512, 256]
    wconvP = np.zeros((128, 8 * 1024), np.float64)
    for dcol, d in enumerate("fb"):
        cw = inputs[f"conv_w_{d}"][:, 0, :][perm].astype(np.float64)
        sw = scales[d]
        for k in range(4):
            tap = cw[:, k] if d == "f" else cw[:, 3 - k]
            WdkT = (tap[:, None] * w_inx).T * sw     # [256, 512]
            for mt in range(4):
                slab = (dcol * 4 + mt) * 1024
                for kt in range(2):
                    off = slab + k * 256 + kt * 128
                    wconvP[:, off:off + 128] = \
                        WdkT[kt * 128:(kt + 1) * 128,
                             mt * 128:(mt + 1) * 128]
    out["wconvP8"] = wconvP.astype(f8)

    wz = inputs["in_proj_w"][512:1024][own].astype(np.float64)  # [128, 256]
    wzP = np.zeros((128, 256), np.float64)
    for kt in range(2):
        wzP[:, kt * 128:(kt + 1) * 128] = wz.T[kt * 128:(kt + 1) * 128]
    out["wzP8"] = (wzP * scales["z"]).astype(f8)

    out["outWT"] = np.ascontiguousarray(
        inputs["out_proj_w"][:, own].T).astype(np.float16)  # [128, 256]

    xprojP = np.zeros((128, 384), np.float32)
    xpf = inputs["xproj_w_f"][:, perm].T  # [512, 48]
    xpb = inputs["xproj_w_b"][:, perm].T
    for kt in range(4):
        xprojP[:, kt * 96:kt * 96 + 48] = xpf[kt * 128:(kt + 1) * 128]
        xprojP[:, kt * 96 + 48:kt * 96 + 96] = xpb[kt * 128:(kt + 1) * 128]
    out["xprojP"] = xprojP.astype(np.float16)

    out["dtprojp"] = np.ascontiguousarray(np.concatenate(
        [inputs["dtproj_w_f"][own].T, inputs["dtproj_w_b"][own].T],
        axis=1)).astype(np.float16)  # [16, 256]

    wconst = np.zeros((128, 76), np.float32)
    for i, d in enumerate("fb"):
        wconst[:, 16 * i:16 * i + 16] = -np.exp(
            inputs[f"A_log_{d}"][own].astype(np.float64))
        cb = inputs[f"conv_b_{d}"][perm]
        wconst[:, 64 + 4 * i:68 + 4 * i] = cb.reshape(4, 128).T
        wconst[:, 74 + i] = inputs[f"dtproj_b_{d}"][own]
    out["wconst"] = wconst

    out["ident"] = np.eye(128, dtype=np.float32).astype(bf)
    diagD = np.zeros((128, 256), np.float32)
    diagD[:, 0:128] = np.diag(inputs["D_f"][own])
    diagD[:, 128:256] = np.diag(inputs["D_b"][own])
    out["diagD"] = diagD.astype(bf)
    return out


def kernel(**inputs):
    inputs = {k: np.asarray(v) for k, v in inputs.items()}
    scales = prep_scales(inputs)
    descales = {"f": 1.0 / (S_X * scales["f"]),
                "b": 1.0 / (S_X * scales["b"]),
                "z": 1.0 / (S_X * scales["z"])}
    if "nc" not in _CACHE:
        _CACHE["nc"] = build_nc(descales)
    nc = _CACHE["nc"]

    core_ids = list(range(8))
    in_maps = [prep_core_inputs(inputs, core // 4, core % 4, scales)
               for core in core_ids]
    trace = os.environ.get("BASS_KERNEL_TRACE", "0") == "1"
    res = run_bass_kernel_spmd(nc, in_maps, core_ids, trace=trace)
    _CACHE["last_results"] = res

    x = inputs["x"].astype(np.float32)
    out = np.empty((B, L, 256), np.float32)
    for b in range(B):
        acc = np.zeros((256, L), np.float32)
        for dq in range(4):
            acc += res.results[4 * b + dq]["attnT"]
        out[b] = x[b] + acc.T
    return out.astype(np.float32)
